# revision 2
# baseline (speedup 1.0000x reference)
"""Causal multi-head attention (B=2, S=2048, D=1024, H=16, HD=64) on 8 trn2 cores.

Sharding: 2 heads per core x both batches (head-parallel QKV/attention/out-proj,
Wo h-split => per-core partial outputs, summed on host).

All matmul operands are fp16 (half DMA/SBUF vs fp32, same 1 cyc/row PE rate,
DVE 2x modes); PSUM accumulation, softmax denominators and normalization stay
fp32. Verified rel err vs fp32 reference: 6.8e-4.

Structure (single fused stream, engines balanced):
  - x^T is DMA'd in s-halves; the first q/k/v projection packets interleave
    d-major with the chunk arrivals so the PE tracks the DMA
  - all other projections are single-psum-bank packets pumped into the
    attention stream with deadline barriers ((b, j) block tags)
  - scores pair the two heads at partition bases 0/64 (row tiling, K=64);
    exp on ACT only; causal masking = DVE multiply with 2 static fp16 tiles
    (the affine iota pattern is qj-independent)
  - attnV runs lagged one group behind scores so the PE FIFO never
    head-of-line blocks on the exp
  - V^T via PE transposes (fp16) through the shared work psum ring
  - normalization: den rows copied to SBUF (custom DVE recip ops must not
    read PSUM: partition-addressing corruption observed on HW), packed
    reciprocal_approx_fast + NR, ones-matmul broadcast; the broadcast
    matmuls and out-proj are deferred into the next qj's pump stream
  - gpsimd is avoided except for f32 affine_select/iota at startup
    (partition_broadcast and fp16 gpsimd ucode paths are broken on HW)

PSUM (8 banks): ps_s = scores + first v packet [P,2,NQ]x2 (4), ps_o = attnV
accumulators [HD+1,NQ]x2 (2), ps_w = out-proj / proj packets / V-transposes /
bc broadcasts [P,NQ]x2 (2).
"""

import collections

import numpy as np

import concourse.bass as bass
import concourse.mybir as mybir
import concourse.tile as tile
from concourse import bacc
from concourse.bass_utils import run_bass_kernel_spmd
from concourse.masks import make_identity
from concourse.dve_ops import RECIPROCAL_APPROX_NR

F32 = mybir.dt.float32
F32R = mybir.dt.float32r
F16 = mybir.dt.float16
AF = mybir.ActivationFunctionType

B, S, D, H, HD = 2, 2048, 1024, 16, 64
NCORES = 8
HPC = H // NCORES          # heads per core = 2
HH = HPC * HD              # 128 concat head dims per core
P = 128
DC = D // P                # 8 d-chunks
NQ = 512                   # q tile (psum bank width fp32)
QJ = S // NQ               # 4 q tiles
KC = S // P                # 16 k chunks
GK = 2                     # k-chunks per score group (psum: [128, GK, NQ])

_NC_CACHE = {}
import os
_DEBUG_DUMP = bool(os.environ.get("K2_DEBUG"))
_PBCAST = bool(os.environ.get("K2_PBCAST"))
_POOLMASK = bool(os.environ.get("K2_POOLMASK"))


def _build_nc(with_bias_qkv: bool, with_bias_o: bool, causal: bool):
    key = (with_bias_qkv, with_bias_o, causal)
    if key in _NC_CACHE:
        return _NC_CACHE[key]

    nc = bacc.Bacc("TRN2", target_bir_lowering=False, debug=False)
    xt = nc.dram_tensor("xt", [B, D, S], F16, kind="ExternalInput")
    wq = nc.dram_tensor("wq", [P, DC, HH], F16, kind="ExternalInput")
    wk = nc.dram_tensor("wk", [P, DC, HH], F16, kind="ExternalInput")
    wv = nc.dram_tensor("wv", [P, DC, HH], F16, kind="ExternalInput")
    wo = nc.dram_tensor("wo", [HH, D], F16, kind="ExternalInput")
    if with_bias_qkv:
        bqkv = nc.dram_tensor("bqkv", [3, HH], F32, kind="ExternalInput")
    if with_bias_o:
        bo8 = nc.dram_tensor("bo8", [D], F16, kind="ExternalInput")
    out = nc.dram_tensor("out", [B, S, D], F16, kind="ExternalOutput")

    with tile.TileContext(nc) as tc:
        with (
            tc.tile_pool(name="const", bufs=1) as cpool,
            tc.tile_pool(name="xtp", bufs=2) as xt_pool,
            tc.tile_pool(name="qkv", bufs=2) as qkv_pool,
            tc.tile_pool(name="otn", bufs=2) as otn_pool,
            tc.tile_pool(name="nrm", bufs=4) as nrm_pool,
            tc.tile_pool(name="ep", bufs=6) as e_pool,
            tc.tile_pool(name="osb", bufs=3) as osb_pool,
            tc.tile_pool(name="ps_s", bufs=2, space="PSUM") as ps_s,
            tc.tile_pool(name="ps_o", bufs=2, space="PSUM") as ps_o,
            tc.tile_pool(name="ps_w", bufs=2, space="PSUM") as ps_w,
        ):
            # ---- constants / weights (DMA order matters: wq then xt b0) ----
            wq_sb = cpool.tile([P, DC, HH], F16, tag="wq", name="wq_sb")
            wk_sb = cpool.tile([P, DC, HH], F16, tag="wk", name="wk_sb")
            wv_sb = cpool.tile([P, DC, HH], F16, tag="wv", name="wv_sb")
            wo_sb = cpool.tile([HH, D], F16, tag="wo", name="wo_sb")
            nc.sync.dma_start(wq_sb[:], wq[:])

            xt_sbs = []

            def alloc_xt(b):
                xt_sb = xt_pool.tile([P, DC, S], F16, tag="xt", name="xt_sb")
                xt_sbs.append(xt_sb)

            def load_xt_half(b, half):
                s0, s1 = half * (S // 2), (half + 1) * (S // 2)
                for d in range(DC):
                    nc.sync.dma_start(xt_sbs[b][:, d, s0:s1],
                                      xt[b, d * P:(d + 1) * P, s0:s1])

            alloc_xt(0)
            alloc_xt(1)
            nc.sync.dma_start(wk_sb[:], wk[:])
            load_xt_half(0, 0)
            nc.sync.dma_start(wv_sb[:], wv[:])
            load_xt_half(0, 1)
            load_xt_half(1, 0)
            load_xt_half(1, 1)
            nc.sync.dma_start(wo_sb[:], wo[:])
            if with_bias_qkv:
                bqkvt_sb = cpool.tile([HH, 3], F32, tag="bqkvt", name="bqkvt_sb")
                for i in range(3):
                    nc.sync.dma_start(
                        bqkvt_sb[:, i:i + 1], bqkv[i:i + 1, :].rearrange("a f -> f a")
                    )
            if with_bias_o:
                bo8_sb = cpool.tile([1, D], F16, tag="bo8", name="bo8_sb")
                nc.sync.dma_start(bo8_sb[:], bo8.rearrange("(a d) -> a d", a=1))
                ones16_sb = cpool.tile([1, P], F16, tag="ones16", name="ones16_sb")
                nc.vector.memset(ones16_sb[:], 1.0)

            ident32_sb = cpool.tile([P, P], F32, tag="ident32", name="ident32_sb")
            make_identity(nc, ident32_sb[:])
            ident_sb = cpool.tile([P, P], F16, tag="ident", name="ident_sb")
            nc.vector.tensor_copy(ident_sb[:], ident32_sb[:])

            # ones for the denominator broadcast matmul (f32r bits == f32);
            # full [P, P] so lhsT slices exist at base partitions 0 and 32
            ones_sb = cpool.tile([P, P], F32R, tag="ones", name="ones_sb")
            nc.vector.memset(ones_sb[:].bitcast(F32), 1.0)

            # causal mask tiles: iota(p, c2, col) = base - P*c2 + col - p >= 0
            # keeps q >= k. For diagonal group g in {2qj, 2qj+1} the base is
            # qj*NQ - g*GK*P = 0 (even g) or -2*P (odd g): qj-independent.
            # Generated in f32 (gpsimd ucode path proven for 32-bit only),
            # then converted to f16 on DVE.
            masks = []
            if causal:
                m32 = cpool.tile([P, GK, NQ], F32, tag="mask32", name="mask32_sb")
                for i in range(2):
                    nc.vector.memset(m32[:], 1.0)
                    nc.gpsimd.affine_select(
                        out=m32[:], in_=m32[:],
                        compare_op=mybir.AluOpType.is_ge, fill=0.0,
                        base=-2 * P * i,
                        pattern=[[-P, GK], [1, NQ]],
                        channel_multiplier=-1,
                    )
                    m = cpool.tile([P, GK, NQ], F16, tag=f"mask{i}",
                                   name=f"mask{i}_sb")
                    nc.vector.tensor_copy(m[:], m32[:])
                    masks.append(m)

            # per-batch state
            st = [dict() for _ in range(B)]

            # alternating engine for psum->sbuf copies
            _alt = [0]

            def copy_alt(dst, src):
                _alt[0] ^= 1
                if _alt[0]:
                    nc.scalar.copy(dst, src)
                else:
                    nc.vector.tensor_copy(dst, src)

            def copy_dve(dst, src):
                nc.vector.tensor_copy(dst, src)

            def proj_copy(w_idx, dst_ap, pps_ap, engine):
                if with_bias_qkv:
                    nc.scalar.activation(
                        dst_ap, pps_ap, AF.Identity,
                        bias=bqkvt_sb[:, w_idx:w_idx + 1],
                    )
                elif engine == "act":
                    nc.scalar.copy(dst_ap, pps_ap)
                elif engine == "dve":
                    nc.vector.tensor_copy(dst_ap, pps_ap)
                else:
                    copy_alt(dst_ap, pps_ap)

            def alloc_qkv(b):
                st[b]["qt"] = qkv_pool.tile([P, QJ, NQ], F16, tag="qt", name="qt_sb")
                st[b]["kt"] = qkv_pool.tile([P, QJ, NQ], F16, tag="kt", name="kt_sb")
                st[b]["vt"] = qkv_pool.tile([P, QJ, NQ], F16, tag="vt", name="vt_sb")
                v_sb = qkv_pool.tile([P, KC, HPC, HD + 1], F16, tag="v", name="v_sb")
                nc.vector.memset(v_sb[:, :, :, HD:], 1.0)
                st[b]["v"] = v_sb

            def proj_packet(b, w_idx, j, engine="alt"):
                # single-bank d-inner packet through the ps_w ring
                w_sb = (wq_sb, wk_sb, wv_sb)[w_idx]
                dst = (st[b]["qt"], st[b]["kt"], st[b]["vt"])[w_idx]
                pp = ps_w.tile([P, NQ], F32, tag="w", name="proj_pp")
                for d in range(DC):
                    nc.tensor.matmul(
                        pp[:],
                        lhsT=w_sb[:, d, :],
                        rhs=xt_sbs[b][:, d, j * NQ:(j + 1) * NQ],
                        start=(d == 0), stop=(d == DC - 1),
                    )
                proj_copy(w_idx, dst[:, j, :], pp[:], engine)

            def v_transpose(b, sc):
                tp = ps_w.tile([P, P], F16, tag="w", name="tr_ps")
                nc.tensor.transpose(
                    tp[:], st[b]["vt"][:, sc // 4, (sc % 4) * P:(sc % 4 + 1) * P],
                    ident_sb[:],
                )
                # single strided copy: tp[:, h*64+d] -> v[:, sc, h, d]
                nc.scalar.copy(
                    st[b]["v"][:, sc, :, :HD],
                    tp.rearrange("p (h d) -> p h d", h=HPC),
                )

            def attn_alloc(b):
                st[b]["otn"] = otn_pool.tile([P, QJ, NQ], F16, tag="otn",
                                             name="otn_sb")

            def attn_qj(b, qj, pump):
                """scores/exp for group g; attnV lagged one group."""
                qt, kt, v = st[b]["qt"], st[b]["kt"], st[b]["v"]
                ngroups = (2 * (qj + 1)) if causal else (KC // GK)
                o_ps = [ps_o.tile([HD + 1, NQ], F32, tag="o", name=f"o_ps{h}")
                        for h in range(HPC)]

                def attn_v(g, e_tiles):
                    for h in range(HPC):
                        for c2 in range(GK):
                            ki = g * GK + c2
                            nc.tensor.matmul(
                                o_ps[h][:],
                                lhsT=v[:, ki, h, :],
                                rhs=e_tiles[h][:, c2, :],
                                start=(g == 0 and c2 == 0),
                                stop=(g == ngroups - 1 and c2 == GK - 1),
                            )

                prev = None
                for g in range(ngroups):
                    stp = []
                    for h in range(HPC):
                        h0 = h * HD
                        sp = ps_s.tile([P, GK, NQ], F32, tag="s", name=f"st_ps{h}")
                        stp.append(sp)
                        for c2 in range(GK):
                            ki = g * GK + c2
                            nc.tensor.matmul(
                                sp[:, c2, :],
                                lhsT=kt[h0:h0 + HD, ki // 4, (ki % 4) * P:(ki % 4 + 1) * P],
                                rhs=qt[h0:h0 + HD, qj, :],
                                start=True, stop=True,
                            )
                    e_tiles = []
                    for h in range(HPC):
                        e_sb = e_pool.tile([P, GK, NQ], F16, tag="e", name="e_sb")
                        nc.scalar.activation(e_sb[:], stp[h][:], AF.Exp, scale=0.125)
                        if causal and g >= 2 * qj:
                            eng = nc.gpsimd if _POOLMASK else nc.vector
                            eng.tensor_mul(e_sb[:], e_sb[:],
                                           masks[g - 2 * qj][:])
                        e_tiles.append(e_sb)
                    if prev is not None:
                        attn_v(*prev)
                    prev = (g, e_tiles)
                    pump()
                attn_v(*prev)
                return o_ps

            def norm_qj(b, qj, o_ps):
                # den rows copied to SBUF first (custom DVE recip ops must NOT
                # read PSUM: observed partition-addressing corruption on HW),
                # both heads packed at partitions 0/32, then per-head
                # ones-matmul broadcast, sbuf copy, and scale into otn (f16)
                otn = st[b]["otn"]
                rsq = nrm_pool.tile([33, NQ], F32, tag="rsq", name="rsq")
                ots = []
                for h in range(HPC):
                    # den-row + o copies first: each head's psum accumulator
                    # releases after two DVE ops, before the recip chain
                    nc.vector.tensor_copy(
                        rsq[32 * h:32 * h + 1, :], o_ps[h][HD:HD + 1, :])
                    ot = nrm_pool.tile([HD, NQ], F32, tag="ot", name="ot")
                    nc.vector.tensor_copy(ot[:], o_ps[h][:HD, :])
                    ots.append(ot)
                rscq = nrm_pool.tile([33, NQ], F32, tag="rscq", name="rscq")
                nc.vector.reciprocal_approx_fast(out=rscq[:], in_=rsq[:])
                rinq = nrm_pool.tile([33, NQ], F32R, tag="rinq", name="rinq")
                nc.vector._custom_dve(
                    RECIPROCAL_APPROX_NR, out=rinq[:], in0=rsq[:], in1=rscq[:],
                    s0=2.0,
                )
                def bc_mul(h):
                    bc_ps = ps_w.tile([HD, NQ], F32, tag="w", name="bc_ps")
                    nc.tensor.matmul(
                        bc_ps[:], lhsT=ones_sb[32 * h:32 * h + 1, :HD],
                        rhs=rinq[32 * h:32 * h + 1, :], start=True, stop=True,
                    )
                    nc.vector.tensor_mul(
                        otn[h * HD:(h + 1) * HD, qj, :], ots[h][:], bc_ps[:])
                return [lambda h=h: bc_mul(h) for h in range(HPC)]

            def outproj_sc(b, sc, engine="alt"):
                qj, sc4 = sc // 4, sc % 4
                otn = st[b]["otn"]
                out_sb = osb_pool.tile([P, 2, NQ], F16, tag="out", name="out_sb")
                for fc in range(2):
                    ops = ps_w.tile([P, NQ], F32, tag="w", name="op_ps")
                    if with_bias_o:
                        nc.tensor.matmul(
                            ops[:], lhsT=ones16_sb[:],
                            rhs=bo8_sb[:, fc * NQ:(fc + 1) * NQ],
                            start=True, stop=False,
                        )
                    nc.tensor.matmul(
                        ops[:],
                        lhsT=otn[:, qj, sc4 * P:(sc4 + 1) * P],
                        rhs=wo_sb[:, fc * NQ:(fc + 1) * NQ],
                        start=not with_bias_o, stop=True,
                    )
                    if engine == "act":
                        nc.scalar.copy(out_sb[:, fc, :], ops[:])
                    else:
                        copy_dve(out_sb[:, fc, :], ops[:])
                nc.sync.dma_start(
                    out[b, sc * P:(sc + 1) * P, :],
                    out_sb.rearrange("p a n -> p (a n)"),
                )

            # ---------------- program ----------------
            alloc_qkv(0)
            alloc_qkv(1)

            # all proj/transpose work as packet blocks, pumped into the
            # attention stream with deadline barriers ((b, j) block tags).
            pre = collections.deque()
            post = collections.deque()  # deferred out-proj

            def add_block(b, j):
                for w_idx in range(3):
                    pre.append(((b, j),
                                lambda bb=b, w=w_idx, jj=j: proj_packet(bb, w, jj)))
                for sc in range(j * 4, (j + 1) * 4):
                    pre.append(((b, j), lambda bb=b, s=sc: v_transpose(bb, s)))

            # block (0,0) inline, d-major interleaved across q/k/v so the PE
            # tracks the xt half0 DMA chunk arrivals
            pq = ps_w.tile([P, NQ], F32, tag="w", name="pp_q0")
            pk = ps_w.tile([P, NQ], F32, tag="w", name="pp_k0")
            pv = ps_s.tile([P, 2, NQ], F32, tag="s", name="pp_v0")
            for d in range(DC):
                for pp, w_sb in ((pq, wq_sb), (pk, wk_sb), (pv, wv_sb)):
                    dst = pp[:, 0, :] if pp is pv else pp[:]
                    nc.tensor.matmul(
                        dst, lhsT=w_sb[:, d, :], rhs=xt_sbs[0][:, d, 0:NQ],
                        start=(d == 0), stop=(d == DC - 1),
                    )
            proj_copy(0, st[0]["qt"][:, 0, :], pq[:], "act")
            proj_copy(1, st[0]["kt"][:, 0, :], pk[:], "act")
            proj_copy(2, st[0]["vt"][:, 0, :], pv[:, 0, :], "act")
            for sc in range(4):
                v_transpose(0, sc)

            for j in range(1, QJ):
                add_block(0, j)
            for j in range(QJ):
                add_block(1, j)

            groups_left = [2 * B * sum(range(1, QJ + 1)) if causal
                           else B * (KC // GK)]
            _ptog = [0]

            def pump():
                groups_left[0] -= 1
                backlog = len(pre) + len(post)
                n = 2 if backlog > groups_left[0] else 1
                for _ in range(n):
                    _ptog[0] ^= 1
                    if pre and (_ptog[0] or not post):
                        pre.popleft()[1]()
                    elif post:
                        post.popleft()()

            def flush_pre(b, qj):
                while pre and pre[0][0][0] == b and pre[0][0][1] <= qj:
                    pre.popleft()[1]()

            attn_alloc(0)
            attn_alloc(1)
            sched = [(0, 0), (0, 1), (0, 2), (0, 3), (1, 0), (1, 1), (1, 2),
                     (1, 3)]
            for i, (b, qj) in enumerate(sched):
                flush_pre(b, qj)
                o_ps = attn_qj(b, qj, pump)
                tail = norm_qj(b, qj, o_ps)
                if i < len(sched) - 1:
                    for fn in tail:
                        post.append(fn)
                    for sc in range(qj * 4, (qj + 1) * 4):
                        post.append(lambda bb=b, s=sc: outproj_sc(bb, s))
                else:
                    for fn in tail:
                        fn()

            while pre:
                pre.popleft()[1]()
            while post:
                post.popleft()()
            for sc in range(12, 16):
                outproj_sc(1, sc, engine="act")

            if _DEBUG_DUMP:
                dbg = nc.dram_tensor("dbg", [B, 4, P, QJ, NQ], F16,
                                     kind="ExternalOutput")
                for b in range(B):
                    nc.sync.dma_start(dbg[b, 0], st[b]["qt"][:])
                    nc.sync.dma_start(dbg[b, 1], st[b]["kt"][:])
                    nc.sync.dma_start(dbg[b, 2], st[b]["vt"][:])
                    nc.sync.dma_start(dbg[b, 3], st[b]["otn"][:])

    nc.compile()
    _NC_CACHE[key] = nc
    return nc


def _check_causal(mask: np.ndarray) -> bool:
    m = np.asarray(mask).reshape(mask.shape[-2], mask.shape[-1])
    s = m.shape[0]
    if np.array_equal(m, np.tril(np.ones((s, s), dtype=bool))):
        return True
    if m.all():
        return False
    raise NotImplementedError("only causal or all-true masks are supported")


def _warr(w):
    # [D, HH] -> [P, DC, HH] fp16 (partition-major chunks, contiguous DMA)
    return np.ascontiguousarray(
        w.reshape(DC, P, HH).transpose(1, 0, 2).astype(np.float16))


def kernel(inputs_q, mask, Wq, bq, Wk, bk, Wv, bv, Wo, bo, _trace=False,
           _trace_cores=None):
    inputs_q = np.asarray(inputs_q, dtype=np.float32)
    Wq = np.asarray(Wq, dtype=np.float32).reshape(D, H * HD)
    Wk = np.asarray(Wk, dtype=np.float32).reshape(D, H * HD)
    Wv = np.asarray(Wv, dtype=np.float32).reshape(D, H * HD)
    Wo = np.asarray(Wo, dtype=np.float32).reshape(H * HD, D)
    bq = np.asarray(bq, dtype=np.float32).reshape(H * HD)
    bk = np.asarray(bk, dtype=np.float32).reshape(H * HD)
    bv = np.asarray(bv, dtype=np.float32).reshape(H * HD)
    bo = np.asarray(bo, dtype=np.float32).reshape(D)

    causal = _check_causal(mask)
    with_bias_qkv = bool(bq.any() or bk.any() or bv.any())
    with_bias_o = bool(bo.any())

    nc = _build_nc(with_bias_qkv, with_bias_o, causal)

    xt = np.ascontiguousarray(
        inputs_q.transpose(0, 2, 1).astype(np.float16))  # [B, D, S]
    in_maps = []
    for c in range(NCORES):
        f0, f1 = c * HH, (c + 1) * HH
        m = {
            "xt": xt,
            "wq": _warr(Wq[:, f0:f1]),
            "wk": _warr(Wk[:, f0:f1]),
            "wv": _warr(Wv[:, f0:f1]),
            "wo": np.ascontiguousarray(Wo[f0:f1, :].astype(np.float16)),
        }
        if with_bias_qkv:
            m["bqkv"] = np.ascontiguousarray(
                np.stack([bq[f0:f1], bk[f0:f1], bv[f0:f1]])
            )
        if with_bias_o:
            m["bo8"] = np.ascontiguousarray((bo / NCORES).astype(np.float16))
        in_maps.append(m)

    kwargs = {}
    if _trace:
        kwargs["trace"] = True
        if _trace_cores is not None:
            kwargs["trace_cores"] = _trace_cores
    res = run_bass_kernel_spmd(nc, in_maps, core_ids=list(range(NCORES)), **kwargs)

    acc = np.zeros((B, S, D), dtype=np.float32)
    for c in range(NCORES):
        acc += res.results[c]["out"].astype(np.float32)
    if not with_bias_o:
        acc += bo  # bo is zero here, but keep the math explicit
    out = acc.astype(np.float32)
    if _trace:
        return out, res
    return out


# revision 4
# speedup vs baseline: 1.0270x; 1.0270x over previous
"""Causal multi-head attention (B=2, S=2048, D=1024, H=16, HD=64) on 8 trn2 cores.

Sharding: 2 heads per core x both batches (head-parallel QKV/attention/out-proj,
Wo h-split => per-core partial outputs, summed on host).

All matmul operands are fp16 (half DMA/SBUF vs fp32, same 1 cyc/row PE rate,
DVE 2x modes); PSUM accumulation, softmax denominators and normalization stay
fp32. Verified on HW: rel err 6.8e-4 vs the fp32 reference.

Structure (single fused stream, engines balanced):
  - x^T is DMA'd in s-halves; the first q/k/v projection packets interleave
    d-major with the chunk arrivals so the PE tracks the DMA
  - all other projections are single-psum-bank packets pumped into the
    attention stream with deadline barriers ((b, j) block tags)
  - scores pair the two heads at partition bases 0/64 (row tiling, K=64);
    exp on ACT only; causal masking = DVE multiply with 2 static fp16 tiles
    (the affine iota pattern is qj-independent)
  - attnV runs lagged one group behind scores so the PE FIFO never
    head-of-line blocks on the exp
  - V^T via PE fp16 transposes through the shared work psum ring
  - normalization: den rows copied to SBUF (custom DVE recip ops must NOT
    read PSUM: partition-addressing corruption observed on HW), packed
    recip chain at partitions 0/32, two accumulating K=1 E-pattern matmuls
    broadcast both heads' reciprocals into one [128,NQ] psum tile, one
    whole-width multiply; broadcast+out-proj deferred into the pump stream
  - gpsimd only does f32 memset/affine_select (partition_broadcast and
    fp16 gpsimd ucode paths produce NaN on this HW)

PSUM (8 banks): ps_s = scores + first v packet [P,2,NQ]x2 (4), ps_o = attnV
accumulators [HD+1,NQ]x2 (2), ps_w = out-proj / proj packets / V-transposes /
bc broadcasts [P,NQ]x2 (2).
"""

import collections

import numpy as np

import concourse.bass as bass
import concourse.mybir as mybir
import concourse.tile as tile
from concourse import bacc
from concourse.bass_utils import run_bass_kernel_spmd
from concourse.masks import make_identity
from concourse.dve_ops import RECIPROCAL_APPROX_NR

F32 = mybir.dt.float32
F32R = mybir.dt.float32r
F16 = mybir.dt.float16
AF = mybir.ActivationFunctionType

B, S, D, H, HD = 2, 2048, 1024, 16, 64
NCORES = 8
HPC = H // NCORES          # heads per core = 2
HH = HPC * HD              # 128 concat head dims per core
P = 128
DC = D // P                # 8 d-chunks
NQ = 512                   # q tile (psum bank width fp32)
QJ = S // NQ               # 4 q tiles
KC = S // P                # 16 k chunks
GK = 2                     # k-chunks per score group (psum: [128, GK, NQ])

_NC_CACHE = {}


def _build_nc(with_bias_qkv: bool, with_bias_o: bool, causal: bool):
    key = (with_bias_qkv, with_bias_o, causal)
    if key in _NC_CACHE:
        return _NC_CACHE[key]

    nc = bacc.Bacc("TRN2", target_bir_lowering=False, debug=False)
    xt = nc.dram_tensor("xt", [B, D, S], F16, kind="ExternalInput")
    wq = nc.dram_tensor("wq", [P, DC, HH], F16, kind="ExternalInput")
    wk = nc.dram_tensor("wk", [P, DC, HH], F16, kind="ExternalInput")
    wv = nc.dram_tensor("wv", [P, DC, HH], F16, kind="ExternalInput")
    wo = nc.dram_tensor("wo", [HH, D], F16, kind="ExternalInput")
    if with_bias_qkv:
        bqkv = nc.dram_tensor("bqkv", [3, HH], F32, kind="ExternalInput")
    if with_bias_o:
        bo8 = nc.dram_tensor("bo8", [D], F16, kind="ExternalInput")
    out = nc.dram_tensor("out", [B, S, D], F16, kind="ExternalOutput")

    with tile.TileContext(nc) as tc:
        with (
            tc.tile_pool(name="const", bufs=1) as cpool,
            tc.tile_pool(name="xtp", bufs=2) as xt_pool,
            tc.tile_pool(name="qkv", bufs=2) as qkv_pool,
            tc.tile_pool(name="otn", bufs=2) as otn_pool,
            tc.tile_pool(name="nrm", bufs=4) as nrm_pool,
            tc.tile_pool(name="ep", bufs=6) as e_pool,
            tc.tile_pool(name="osb", bufs=3) as osb_pool,
            tc.tile_pool(name="ps_s", bufs=2, space="PSUM") as ps_s,
            tc.tile_pool(name="ps_o", bufs=2, space="PSUM") as ps_o,
            tc.tile_pool(name="ps_w", bufs=2, space="PSUM") as ps_w,
        ):
            # ---- constants / weights (DMA order matters: wq then xt b0) ----
            wq_sb = cpool.tile([P, DC, HH], F16, tag="wq", name="wq_sb")
            wk_sb = cpool.tile([P, DC, HH], F16, tag="wk", name="wk_sb")
            wv_sb = cpool.tile([P, DC, HH], F16, tag="wv", name="wv_sb")
            wo_sb = cpool.tile([HH, D], F16, tag="wo", name="wo_sb")
            nc.sync.dma_start(wq_sb[:], wq[:])

            xt_sbs = []

            def alloc_xt(b):
                xt_sb = xt_pool.tile([P, DC, S], F16, tag="xt", name="xt_sb")
                xt_sbs.append(xt_sb)

            def load_xt_half(b, half):
                s0, s1 = half * (S // 2), (half + 1) * (S // 2)
                for d in range(DC):
                    nc.sync.dma_start(xt_sbs[b][:, d, s0:s1],
                                      xt[b, d * P:(d + 1) * P, s0:s1])

            alloc_xt(0)
            alloc_xt(1)
            nc.sync.dma_start(wk_sb[:], wk[:])
            load_xt_half(0, 0)
            nc.sync.dma_start(wv_sb[:], wv[:])
            load_xt_half(0, 1)
            load_xt_half(1, 0)
            load_xt_half(1, 1)
            nc.sync.dma_start(wo_sb[:], wo[:])
            if with_bias_qkv:
                bqkvt_sb = cpool.tile([HH, 3], F32, tag="bqkvt", name="bqkvt_sb")
                for i in range(3):
                    nc.sync.dma_start(
                        bqkvt_sb[:, i:i + 1], bqkv[i:i + 1, :].rearrange("a f -> f a")
                    )
            if with_bias_o:
                bo8_sb = cpool.tile([1, D], F16, tag="bo8", name="bo8_sb")
                nc.sync.dma_start(bo8_sb[:], bo8.rearrange("(a d) -> a d", a=1))
                ones16_sb = cpool.tile([1, P], F16, tag="ones16", name="ones16_sb")
                nc.vector.memset(ones16_sb[:], 1.0)

            ident32_sb = cpool.tile([P, P], F32, tag="ident32", name="ident32_sb")
            make_identity(nc, ident32_sb[:])
            ident_sb = cpool.tile([P, P], F16, tag="ident", name="ident_sb")
            nc.vector.tensor_copy(ident_sb[:], ident32_sb[:])

            # E-pattern for the denominator broadcast matmuls (f32r bits ==
            # f32): row at partition 32h covers output partitions 64h..64h+63,
            # so two accumulating K=1 matmuls broadcast both heads'
            # reciprocals into one [128, NQ] psum tile
            epat_sb = cpool.tile([33, P], F32R, tag="epat", name="epat_sb")
            nc.vector.memset(epat_sb[:].bitcast(F32), 0.0)
            nc.vector.memset(epat_sb[0:1, 0:HD].bitcast(F32), 1.0)
            nc.vector.memset(epat_sb[32:33, HD:P].bitcast(F32), 1.0)

            # causal mask tiles: iota(p, c2, col) = base - P*c2 + col - p >= 0
            # keeps q >= k. For diagonal group g in {2qj, 2qj+1} the base is
            # qj*NQ - g*GK*P = 0 (even g) or -2*P (odd g): qj-independent.
            # Generated in f32 (gpsimd ucode path proven for 32-bit only),
            # then converted to f16 on DVE.
            masks = []
            if causal:
                m32 = cpool.tile([P, GK, NQ], F32, tag="mask32", name="mask32_sb")
                for i in range(2):
                    nc.vector.memset(m32[:], 1.0)
                    nc.gpsimd.affine_select(
                        out=m32[:], in_=m32[:],
                        compare_op=mybir.AluOpType.is_ge, fill=0.0,
                        base=-2 * P * i,
                        pattern=[[-P, GK], [1, NQ]],
                        channel_multiplier=-1,
                    )
                    m = cpool.tile([P, GK, NQ], F16, tag=f"mask{i}",
                                   name=f"mask{i}_sb")
                    nc.vector.tensor_copy(m[:], m32[:])
                    masks.append(m)

            # per-batch state
            st = [dict() for _ in range(B)]

            # alternating engine for psum->sbuf copies
            _alt = [0]

            def copy_alt(dst, src):
                _alt[0] ^= 1
                if _alt[0]:
                    nc.scalar.copy(dst, src)
                else:
                    nc.vector.tensor_copy(dst, src)

            def copy_dve(dst, src):
                nc.vector.tensor_copy(dst, src)

            def proj_copy(w_idx, dst_ap, pps_ap, engine):
                if with_bias_qkv:
                    nc.scalar.activation(
                        dst_ap, pps_ap, AF.Identity,
                        bias=bqkvt_sb[:, w_idx:w_idx + 1],
                    )
                elif engine == "act":
                    nc.scalar.copy(dst_ap, pps_ap)
                elif engine == "dve":
                    nc.vector.tensor_copy(dst_ap, pps_ap)
                else:
                    copy_alt(dst_ap, pps_ap)

            def alloc_qkv(b):
                st[b]["qt"] = qkv_pool.tile([P, QJ, NQ], F16, tag="qt", name="qt_sb")
                st[b]["kt"] = qkv_pool.tile([P, QJ, NQ], F16, tag="kt", name="kt_sb")
                st[b]["vt"] = qkv_pool.tile([P, QJ, NQ], F16, tag="vt", name="vt_sb")
                v_sb = qkv_pool.tile([P, KC, HPC, HD + 1], F16, tag="v", name="v_sb")
                nc.vector.memset(v_sb[:, :, :, HD:], 1.0)
                st[b]["v"] = v_sb

            def proj_packet(b, w_idx, j, engine="alt"):
                # single-bank d-inner packet through the ps_w ring
                w_sb = (wq_sb, wk_sb, wv_sb)[w_idx]
                dst = (st[b]["qt"], st[b]["kt"], st[b]["vt"])[w_idx]
                pp = ps_w.tile([P, NQ], F32, tag="w", name="proj_pp")
                for d in range(DC):
                    nc.tensor.matmul(
                        pp[:],
                        lhsT=w_sb[:, d, :],
                        rhs=xt_sbs[b][:, d, j * NQ:(j + 1) * NQ],
                        start=(d == 0), stop=(d == DC - 1),
                    )
                proj_copy(w_idx, dst[:, j, :], pp[:], engine)

            def v_transpose(b, sc):
                tp = ps_w.tile([P, P], F16, tag="w", name="tr_ps")
                nc.tensor.transpose(
                    tp[:], st[b]["vt"][:, sc // 4, (sc % 4) * P:(sc % 4 + 1) * P],
                    ident_sb[:],
                )
                # single strided copy: tp[:, h*64+d] -> v[:, sc, h, d]
                nc.scalar.copy(
                    st[b]["v"][:, sc, :, :HD],
                    tp.rearrange("p (h d) -> p h d", h=HPC),
                )

            def attn_alloc(b):
                st[b]["otn"] = otn_pool.tile([P, QJ, NQ], F16, tag="otn",
                                             name="otn_sb")

            def attn_qj(b, qj, pump):
                """scores/exp for group g; attnV lagged one group."""
                qt, kt, v = st[b]["qt"], st[b]["kt"], st[b]["v"]
                ngroups = (2 * (qj + 1)) if causal else (KC // GK)
                o_ps = [ps_o.tile([HD + 1, NQ], F32, tag="o", name=f"o_ps{h}")
                        for h in range(HPC)]

                def attn_v(g, e_tiles):
                    for h in range(HPC):
                        for c2 in range(GK):
                            ki = g * GK + c2
                            nc.tensor.matmul(
                                o_ps[h][:],
                                lhsT=v[:, ki, h, :],
                                rhs=e_tiles[h][:, c2, :],
                                start=(g == 0 and c2 == 0),
                                stop=(g == ngroups - 1 and c2 == GK - 1),
                            )

                prev = None
                for g in range(ngroups):
                    stp = []
                    for h in range(HPC):
                        h0 = h * HD
                        sp = ps_s.tile([P, GK, NQ], F32, tag="s", name=f"st_ps{h}")
                        stp.append(sp)
                        for c2 in range(GK):
                            ki = g * GK + c2
                            nc.tensor.matmul(
                                sp[:, c2, :],
                                lhsT=kt[h0:h0 + HD, ki // 4, (ki % 4) * P:(ki % 4 + 1) * P],
                                rhs=qt[h0:h0 + HD, qj, :],
                                start=True, stop=True,
                            )
                    e_tiles = []
                    for h in range(HPC):
                        e_sb = e_pool.tile([P, GK, NQ], F16, tag="e", name="e_sb")
                        nc.scalar.activation(e_sb[:], stp[h][:], AF.Exp, scale=0.125)
                        if causal and g >= 2 * qj:
                            nc.vector.tensor_mul(e_sb[:], e_sb[:],
                                                 masks[g - 2 * qj][:])
                        e_tiles.append(e_sb)
                    if prev is not None:
                        attn_v(*prev)
                    prev = (g, e_tiles)
                    pump()
                attn_v(*prev)
                return o_ps

            def norm_qj(b, qj, o_ps):
                # den rows copied to SBUF first (custom DVE recip ops must NOT
                # read PSUM: observed partition-addressing corruption on HW),
                # both heads packed at partitions 0/1, one recip chain, then
                # one E-pattern broadcast matmul + one whole-width scale
                otn = st[b]["otn"]
                rsq = nrm_pool.tile([33, NQ], F32, tag="rsq", name="rsq")
                # partitions 1-31 are never consumed (epat matmuls read only
                # rows 0/32) but the packed recip ops scan them: give the tile
                # a benign fill on the idle Pool engine before the den copies
                nc.gpsimd.memset(rsq[:, :], 1.0)
                otb = nrm_pool.tile([P, NQ], F32, tag="ot", name="otb")
                for h in range(HPC):
                    # den-row + o copies first: each head's psum accumulator
                    # releases after two DVE ops, before the recip chain
                    nc.vector.tensor_copy(
                        rsq[32 * h:32 * h + 1, :], o_ps[h][HD:HD + 1, :])
                    nc.vector.tensor_copy(
                        otb[h * HD:(h + 1) * HD, :], o_ps[h][:HD, :])
                rscq = nrm_pool.tile([33, NQ], F32, tag="rscq", name="rscq")
                nc.vector.reciprocal_approx_fast(out=rscq[:], in_=rsq[:])
                rinq = nrm_pool.tile([33, NQ], F32R, tag="rinq", name="rinq")
                nc.vector._custom_dve(
                    RECIPROCAL_APPROX_NR, out=rinq[:], in0=rsq[:], in1=rscq[:],
                    s0=2.0,
                )
                def bc_mul():
                    # single K=33 matmul: rows 1-31 of rinq are benign (1.0)
                    # and hit zero weights; two K=1 row-tiled matmuls would
                    # accumulate into the same psum region CONCURRENTLY
                    bc_ps = ps_w.tile([P, NQ], F32, tag="w", name="bc_ps")
                    nc.tensor.matmul(
                        bc_ps[:], lhsT=epat_sb[:], rhs=rinq[:],
                        start=True, stop=True,
                    )
                    nc.vector.tensor_mul(otn[:, qj, :], otb[:], bc_ps[:])
                return [bc_mul]

            def outproj_sc(b, sc, engine="alt"):
                qj, sc4 = sc // 4, sc % 4
                otn = st[b]["otn"]
                out_sb = osb_pool.tile([P, 2, NQ], F16, tag="out", name="out_sb")
                for fc in range(2):
                    ops = ps_w.tile([P, NQ], F32, tag="w", name="op_ps")
                    if with_bias_o:
                        nc.tensor.matmul(
                            ops[:], lhsT=ones16_sb[:],
                            rhs=bo8_sb[:, fc * NQ:(fc + 1) * NQ],
                            start=True, stop=False,
                        )
                    nc.tensor.matmul(
                        ops[:],
                        lhsT=otn[:, qj, sc4 * P:(sc4 + 1) * P],
                        rhs=wo_sb[:, fc * NQ:(fc + 1) * NQ],
                        start=not with_bias_o, stop=True,
                    )
                    if engine == "act":
                        nc.scalar.copy(out_sb[:, fc, :], ops[:])
                    else:
                        copy_dve(out_sb[:, fc, :], ops[:])
                nc.sync.dma_start(
                    out[b, sc * P:(sc + 1) * P, :],
                    out_sb.rearrange("p a n -> p (a n)"),
                )

            # ---------------- program ----------------
            alloc_qkv(0)
            alloc_qkv(1)

            # all proj/transpose work as packet blocks, pumped into the
            # attention stream with deadline barriers ((b, j) block tags).
            pre = collections.deque()
            post = collections.deque()  # deferred out-proj

            def add_block(b, j):
                for w_idx in range(3):
                    pre.append(((b, j),
                                lambda bb=b, w=w_idx, jj=j: proj_packet(bb, w, jj)))
                for sc in range(j * 4, (j + 1) * 4):
                    pre.append(((b, j), lambda bb=b, s=sc: v_transpose(bb, s)))

            # block (0,0) inline, d-major interleaved across q/k/v so the PE
            # tracks the xt half0 DMA chunk arrivals
            pq = ps_w.tile([P, NQ], F32, tag="w", name="pp_q0")
            pk = ps_w.tile([P, NQ], F32, tag="w", name="pp_k0")
            pv = ps_s.tile([P, 2, NQ], F32, tag="s", name="pp_v0")
            for d in range(DC):
                for pp, w_sb in ((pq, wq_sb), (pk, wk_sb), (pv, wv_sb)):
                    dst = pp[:, 0, :] if pp is pv else pp[:]
                    nc.tensor.matmul(
                        dst, lhsT=w_sb[:, d, :], rhs=xt_sbs[0][:, d, 0:NQ],
                        start=(d == 0), stop=(d == DC - 1),
                    )
            proj_copy(0, st[0]["qt"][:, 0, :], pq[:], "act")
            proj_copy(1, st[0]["kt"][:, 0, :], pk[:], "act")
            proj_copy(2, st[0]["vt"][:, 0, :], pv[:, 0, :], "act")
            for sc in range(4):
                v_transpose(0, sc)

            for j in range(1, QJ):
                add_block(0, j)
            for j in range(QJ):
                add_block(1, j)

            groups_left = [2 * B * sum(range(1, QJ + 1)) if causal
                           else B * (KC // GK)]
            _ptog = [0]

            def pump():
                groups_left[0] -= 1
                backlog = len(pre) + len(post)
                n = 2 if backlog > groups_left[0] else 1
                for _ in range(n):
                    _ptog[0] ^= 1
                    if pre and (_ptog[0] or not post):
                        pre.popleft()[1]()
                    elif post:
                        post.popleft()()

            def flush_pre(b, qj):
                while pre and pre[0][0][0] == b and pre[0][0][1] <= qj:
                    pre.popleft()[1]()

            attn_alloc(0)
            attn_alloc(1)
            sched = [(0, 0), (0, 1), (0, 2), (0, 3), (1, 0), (1, 1), (1, 2),
                     (1, 3)]
            for i, (b, qj) in enumerate(sched):
                flush_pre(b, qj)
                o_ps = attn_qj(b, qj, pump)
                tail = norm_qj(b, qj, o_ps)
                if i < len(sched) - 1:
                    for fn in tail:
                        post.append(fn)
                    for sc in range(qj * 4, (qj + 1) * 4):
                        post.append(lambda bb=b, s=sc: outproj_sc(bb, s))
                else:
                    for fn in tail:
                        fn()

            while pre:
                pre.popleft()[1]()
            while post:
                post.popleft()()
            for sc in range(12, 16):
                outproj_sc(1, sc, engine="act")

    nc.compile()
    _NC_CACHE[key] = nc
    return nc


def _check_causal(mask: np.ndarray) -> bool:
    m = np.asarray(mask).reshape(mask.shape[-2], mask.shape[-1])
    s = m.shape[0]
    if np.array_equal(m, np.tril(np.ones((s, s), dtype=bool))):
        return True
    if m.all():
        return False
    raise NotImplementedError("only causal or all-true masks are supported")


def _warr(w):
    # [D, HH] -> [P, DC, HH] fp16 (partition-major chunks, contiguous DMA)
    return np.ascontiguousarray(
        w.reshape(DC, P, HH).transpose(1, 0, 2).astype(np.float16))


def kernel(inputs_q, mask, Wq, bq, Wk, bk, Wv, bv, Wo, bo, _trace=False,
           _trace_cores=None):
    inputs_q = np.asarray(inputs_q, dtype=np.float32)
    Wq = np.asarray(Wq, dtype=np.float32).reshape(D, H * HD)
    Wk = np.asarray(Wk, dtype=np.float32).reshape(D, H * HD)
    Wv = np.asarray(Wv, dtype=np.float32).reshape(D, H * HD)
    Wo = np.asarray(Wo, dtype=np.float32).reshape(H * HD, D)
    bq = np.asarray(bq, dtype=np.float32).reshape(H * HD)
    bk = np.asarray(bk, dtype=np.float32).reshape(H * HD)
    bv = np.asarray(bv, dtype=np.float32).reshape(H * HD)
    bo = np.asarray(bo, dtype=np.float32).reshape(D)

    causal = _check_causal(mask)
    with_bias_qkv = bool(bq.any() or bk.any() or bv.any())
    with_bias_o = bool(bo.any())

    nc = _build_nc(with_bias_qkv, with_bias_o, causal)

    xt = np.ascontiguousarray(
        inputs_q.transpose(0, 2, 1).astype(np.float16))  # [B, D, S]
    in_maps = []
    for c in range(NCORES):
        f0, f1 = c * HH, (c + 1) * HH
        m = {
            "xt": xt,
            "wq": _warr(Wq[:, f0:f1]),
            "wk": _warr(Wk[:, f0:f1]),
            "wv": _warr(Wv[:, f0:f1]),
            "wo": np.ascontiguousarray(Wo[f0:f1, :].astype(np.float16)),
        }
        if with_bias_qkv:
            m["bqkv"] = np.ascontiguousarray(
                np.stack([bq[f0:f1], bk[f0:f1], bv[f0:f1]])
            )
        if with_bias_o:
            m["bo8"] = np.ascontiguousarray((bo / NCORES).astype(np.float16))
        in_maps.append(m)

    kwargs = {}
    if _trace:
        kwargs["trace"] = True
        if _trace_cores is not None:
            kwargs["trace_cores"] = _trace_cores
    res = run_bass_kernel_spmd(nc, in_maps, core_ids=list(range(NCORES)), **kwargs)

    acc = np.zeros((B, S, D), dtype=np.float32)
    for c in range(NCORES):
        acc += res.results[c]["out"].astype(np.float32)
    if not with_bias_o:
        acc += bo  # bo is zero here, but keep the math explicit
    out = acc.astype(np.float32)
    if _trace:
        return out, res
    return out


# revision 5
# speedup vs baseline: 1.0340x; 1.0068x over previous
"""Causal multi-head attention (B=2, S=2048, D=1024, H=16, HD=64) on 8 trn2 cores.

Sharding: 2 heads per core x both batches (head-parallel QKV/attention/out-proj,
Wo h-split => per-core partial outputs, summed on host).

All matmul operands are fp16 (half DMA/SBUF vs fp32, same 1 cyc/row PE rate,
DVE 2x modes); PSUM accumulation, softmax denominators and normalization stay
fp32. Verified on HW: rel err 6.8e-4 vs the fp32 reference.

Structure (single fused stream, engines balanced):
  - x^T is DMA'd in s-halves; the first q/k/v projection packets interleave
    d-major with the chunk arrivals so the PE tracks the DMA
  - all other projections are single-psum-bank packets pumped into the
    attention stream with deadline barriers ((b, j) block tags)
  - scores pair the two heads at partition bases 0/64 (row tiling, K=64);
    exp on ACT only; causal masking = DVE multiply with 2 static fp16 tiles
    (the affine iota pattern is qj-independent)
  - attnV runs lagged one group behind scores so the PE FIFO never
    head-of-line blocks on the exp
  - V^T via PE fp16 transposes through the shared work psum ring
  - normalization: den rows copied to SBUF (custom DVE recip ops must NOT
    read PSUM: partition-addressing corruption observed on HW), packed
    recip chain at partitions 0/32, two accumulating K=1 E-pattern matmuls
    broadcast both heads' reciprocals into one [128,NQ] psum tile, one
    whole-width multiply; broadcast+out-proj deferred into the pump stream
  - gpsimd only does f32 memset/affine_select (partition_broadcast and
    fp16 gpsimd ucode paths produce NaN on this HW)

PSUM (8 banks): ps_s = scores + first v packet [P,2,NQ]x2 (4), ps_o = attnV
accumulators [HD+1,NQ]x2 (2), ps_w = out-proj / proj packets / V-transposes /
bc broadcasts [P,NQ]x2 (2).
"""

import collections

import numpy as np

import concourse.bass as bass
import concourse.mybir as mybir
import concourse.tile as tile
from concourse import bacc
from concourse.bass_utils import run_bass_kernel_spmd
from concourse.masks import make_identity
from concourse.dve_ops import RECIPROCAL_APPROX_NR

F32 = mybir.dt.float32
F32R = mybir.dt.float32r
F16 = mybir.dt.float16
AF = mybir.ActivationFunctionType

B, S, D, H, HD = 2, 2048, 1024, 16, 64
NCORES = 8
HPC = H // NCORES          # heads per core = 2
HH = HPC * HD              # 128 concat head dims per core
P = 128
DC = D // P                # 8 d-chunks
NQ = 512                   # q tile (psum bank width fp32)
QJ = S // NQ               # 4 q tiles
KC = S // P                # 16 k chunks
GK = 2                     # k-chunks per score group (psum: [128, GK, NQ])

_NC_CACHE = {}


def _build_nc(with_bias_qkv: bool, with_bias_o: bool, causal: bool):
    key = (with_bias_qkv, with_bias_o, causal)
    if key in _NC_CACHE:
        return _NC_CACHE[key]

    nc = bacc.Bacc("TRN2", target_bir_lowering=False, debug=False)
    xt = nc.dram_tensor("xt", [B, D, S], F16, kind="ExternalInput")
    wq = nc.dram_tensor("wq", [P, DC, HH], F16, kind="ExternalInput")
    wk = nc.dram_tensor("wk", [P, DC, HH], F16, kind="ExternalInput")
    wv = nc.dram_tensor("wv", [P, DC, HH], F16, kind="ExternalInput")
    wo = nc.dram_tensor("wo", [HH, D], F16, kind="ExternalInput")
    if with_bias_qkv:
        bqkv = nc.dram_tensor("bqkv", [3, HH], F32, kind="ExternalInput")
    if with_bias_o:
        bo8 = nc.dram_tensor("bo8", [D], F16, kind="ExternalInput")
    out = nc.dram_tensor("out", [B, S, D], F16, kind="ExternalOutput")

    with tile.TileContext(nc) as tc:
        with (
            tc.tile_pool(name="const", bufs=1) as cpool,
            tc.tile_pool(name="xtp", bufs=2) as xt_pool,
            tc.tile_pool(name="qkv", bufs=2) as qkv_pool,
            tc.tile_pool(name="otn", bufs=2) as otn_pool,
            tc.tile_pool(name="nrm", bufs=4) as nrm_pool,
            tc.tile_pool(name="ep", bufs=6) as e_pool,
            tc.tile_pool(name="osb", bufs=3) as osb_pool,
            tc.tile_pool(name="ps_s", bufs=2, space="PSUM") as ps_s,
            tc.tile_pool(name="ps_o", bufs=2, space="PSUM") as ps_o,
            tc.tile_pool(name="ps_w", bufs=2, space="PSUM") as ps_w,
        ):
            # ---- constants / weights (DMA order matters: wq then xt b0) ----
            wq_sb = cpool.tile([P, DC, HH], F16, tag="wq", name="wq_sb")
            wk_sb = cpool.tile([P, DC, HH], F16, tag="wk", name="wk_sb")
            wv_sb = cpool.tile([P, DC, HH], F16, tag="wv", name="wv_sb")
            wo_sb = cpool.tile([HH, D], F16, tag="wo", name="wo_sb")
            nc.sync.dma_start(wq_sb[:], wq[:])

            xt_sbs = []

            def alloc_xt(b):
                xt_sb = xt_pool.tile([P, DC, S], F16, tag="xt", name="xt_sb")
                xt_sbs.append(xt_sb)

            def load_xt_half(b, half):
                s0, s1 = half * (S // 2), (half + 1) * (S // 2)
                for d in range(DC):
                    nc.sync.dma_start(xt_sbs[b][:, d, s0:s1],
                                      xt[b, d * P:(d + 1) * P, s0:s1])

            alloc_xt(0)
            alloc_xt(1)
            nc.sync.dma_start(wk_sb[:], wk[:])
            load_xt_half(0, 0)
            nc.sync.dma_start(wv_sb[:], wv[:])
            load_xt_half(0, 1)
            load_xt_half(1, 0)
            load_xt_half(1, 1)
            nc.sync.dma_start(wo_sb[:], wo[:])
            if with_bias_qkv:
                bqkvt_sb = cpool.tile([HH, 3], F32, tag="bqkvt", name="bqkvt_sb")
                for i in range(3):
                    nc.sync.dma_start(
                        bqkvt_sb[:, i:i + 1], bqkv[i:i + 1, :].rearrange("a f -> f a")
                    )
            if with_bias_o:
                bo8_sb = cpool.tile([1, D], F16, tag="bo8", name="bo8_sb")
                nc.sync.dma_start(bo8_sb[:], bo8.rearrange("(a d) -> a d", a=1))
                ones16_sb = cpool.tile([1, P], F16, tag="ones16", name="ones16_sb")
                nc.vector.memset(ones16_sb[:], 1.0)

            ident32_sb = cpool.tile([P, P], F32, tag="ident32", name="ident32_sb")
            make_identity(nc, ident32_sb[:])
            ident_sb = cpool.tile([P, P], F16, tag="ident", name="ident_sb")
            nc.vector.tensor_copy(ident_sb[:], ident32_sb[:])

            # E-pattern for the denominator broadcast matmuls (f32r bits ==
            # f32): row at partition 32h covers output partitions 64h..64h+63,
            # so two accumulating K=1 matmuls broadcast both heads'
            # reciprocals into one [128, NQ] psum tile
            epat_sb = cpool.tile([33, P], F32R, tag="epat", name="epat_sb")
            nc.vector.memset(epat_sb[:].bitcast(F32), 0.0)
            nc.vector.memset(epat_sb[0:1, 0:HD].bitcast(F32), 1.0)
            nc.vector.memset(epat_sb[32:33, HD:P].bitcast(F32), 1.0)

            # causal mask tiles: iota(p, c2, col) = base - P*c2 + col - p >= 0
            # keeps q >= k. For diagonal group g in {2qj, 2qj+1} the base is
            # qj*NQ - g*GK*P = 0 (even g) or -2*P (odd g): qj-independent.
            # Generated in f32 (gpsimd ucode path proven for 32-bit only),
            # then converted to f16 on DVE.
            masks = []
            if causal:
                m32 = cpool.tile([P, GK, NQ], F32, tag="mask32", name="mask32_sb")
                for i in range(2):
                    nc.vector.memset(m32[:], 1.0)
                    nc.gpsimd.affine_select(
                        out=m32[:], in_=m32[:],
                        compare_op=mybir.AluOpType.is_ge, fill=0.0,
                        base=-2 * P * i,
                        pattern=[[-P, GK], [1, NQ]],
                        channel_multiplier=-1,
                    )
                    m = cpool.tile([P, GK, NQ], F16, tag=f"mask{i}",
                                   name=f"mask{i}_sb")
                    nc.vector.tensor_copy(m[:], m32[:])
                    masks.append(m)

            # per-batch state
            st = [dict() for _ in range(B)]

            # alternating engine for psum->sbuf copies
            _alt = [0]

            def copy_alt(dst, src):
                _alt[0] ^= 1
                if _alt[0]:
                    nc.scalar.copy(dst, src)
                else:
                    nc.vector.tensor_copy(dst, src)

            def copy_dve(dst, src):
                nc.vector.tensor_copy(dst, src)

            def proj_copy(w_idx, dst_ap, pps_ap, engine):
                if with_bias_qkv:
                    nc.scalar.activation(
                        dst_ap, pps_ap, AF.Identity,
                        bias=bqkvt_sb[:, w_idx:w_idx + 1],
                    )
                elif engine == "act":
                    nc.scalar.copy(dst_ap, pps_ap)
                elif engine == "dve":
                    nc.vector.tensor_copy(dst_ap, pps_ap)
                else:
                    copy_alt(dst_ap, pps_ap)

            def alloc_qkv(b):
                st[b]["qt"] = qkv_pool.tile([P, QJ, NQ], F16, tag="qt", name="qt_sb")
                st[b]["kt"] = qkv_pool.tile([P, QJ, NQ], F16, tag="kt", name="kt_sb")
                st[b]["vt"] = qkv_pool.tile([P, QJ, NQ], F16, tag="vt", name="vt_sb")
                v_sb = qkv_pool.tile([P, KC, HPC, HD + 1], F16, tag="v", name="v_sb")
                nc.vector.memset(v_sb[:, :, :, HD:], 1.0)
                st[b]["v"] = v_sb

            def proj_packet(b, w_idx, j, engine="alt"):
                # single-bank d-inner packet through the ps_w ring
                w_sb = (wq_sb, wk_sb, wv_sb)[w_idx]
                dst = (st[b]["qt"], st[b]["kt"], st[b]["vt"])[w_idx]
                pp = ps_w.tile([P, NQ], F32, tag="w", name="proj_pp")
                for d in range(DC):
                    nc.tensor.matmul(
                        pp[:],
                        lhsT=w_sb[:, d, :],
                        rhs=xt_sbs[b][:, d, j * NQ:(j + 1) * NQ],
                        start=(d == 0), stop=(d == DC - 1),
                    )
                proj_copy(w_idx, dst[:, j, :], pp[:], engine)

            def v_transpose(b, sc):
                tp = ps_w.tile([P, P], F16, tag="w", name="tr_ps")
                nc.tensor.transpose(
                    tp[:], st[b]["vt"][:, sc // 4, (sc % 4) * P:(sc % 4 + 1) * P],
                    ident_sb[:],
                )
                # single strided copy: tp[:, h*64+d] -> v[:, sc, h, d]
                nc.scalar.copy(
                    st[b]["v"][:, sc, :, :HD],
                    tp.rearrange("p (h d) -> p h d", h=HPC),
                )

            def attn_alloc(b):
                st[b]["otn"] = otn_pool.tile([P, QJ, NQ], F16, tag="otn",
                                             name="otn_sb")

            def attn_qj(b, qj, pump):
                """scores/exp for group g; attnV lagged one group."""
                qt, kt, v = st[b]["qt"], st[b]["kt"], st[b]["v"]
                ngroups = (2 * (qj + 1)) if causal else (KC // GK)
                o_ps = [ps_o.tile([HD + 1, NQ], F32, tag="o", name=f"o_ps{h}")
                        for h in range(HPC)]

                def attn_v(g, e_tiles):
                    for h in range(HPC):
                        for c2 in range(GK):
                            ki = g * GK + c2
                            nc.tensor.matmul(
                                o_ps[h][:],
                                lhsT=v[:, ki, h, :],
                                rhs=e_tiles[h][:, c2, :],
                                start=(g == 0 and c2 == 0),
                                stop=(g == ngroups - 1 and c2 == GK - 1),
                            )

                prev = None
                for g in range(ngroups):
                    stp = []
                    for h in range(HPC):
                        h0 = h * HD
                        sp = ps_s.tile([P, GK, NQ], F32, tag="s", name=f"st_ps{h}")
                        stp.append(sp)
                        for c2 in range(GK):
                            ki = g * GK + c2
                            nc.tensor.matmul(
                                sp[:, c2, :],
                                lhsT=kt[h0:h0 + HD, ki // 4, (ki % 4) * P:(ki % 4 + 1) * P],
                                rhs=qt[h0:h0 + HD, qj, :],
                                start=True, stop=True,
                            )
                    e_tiles = []
                    for h in range(HPC):
                        e_sb = e_pool.tile([P, GK, NQ], F16, tag="e", name="e_sb")
                        nc.scalar.activation(e_sb[:], stp[h][:], AF.Exp, scale=0.125)
                        if causal and g >= 2 * qj:
                            nc.vector.tensor_mul(e_sb[:], e_sb[:],
                                                 masks[g - 2 * qj][:])
                        e_tiles.append(e_sb)
                    if prev is not None:
                        attn_v(*prev)
                    prev = (g, e_tiles)
                    pump()
                attn_v(*prev)
                return o_ps

            def norm_qj(b, qj, o_ps, last=False):
                # den rows copied to SBUF first (custom DVE recip ops must NOT
                # read PSUM: observed partition-addressing corruption on HW),
                # both heads packed at partitions 0/1, one recip chain, then
                # one E-pattern broadcast matmul + one whole-width scale
                otn = st[b]["otn"]
                rsq = nrm_pool.tile([33, NQ], F32, tag="rsq", name="rsq")
                # partitions 1-31 are never consumed (epat matmuls read only
                # rows 0/32) but the packed recip ops scan them: give the tile
                # a benign fill on the idle Pool engine before the den copies
                nc.gpsimd.memset(rsq[:, :], 1.0)
                otb = nrm_pool.tile([P, NQ], F32, tag="ot", name="otb")
                for h in range(HPC):
                    # den-row + o copies first: each head's psum accumulator
                    # releases after two DVE ops, before the recip chain
                    nc.vector.tensor_copy(
                        rsq[32 * h:32 * h + 1, :], o_ps[h][HD:HD + 1, :])
                    if last:
                        # tail: ACT is exp-idle; keep DVE free for the recips
                        nc.scalar.copy(
                            otb[h * HD:(h + 1) * HD, :], o_ps[h][:HD, :])
                    else:
                        nc.vector.tensor_copy(
                            otb[h * HD:(h + 1) * HD, :], o_ps[h][:HD, :])
                rscq = nrm_pool.tile([33, NQ], F32, tag="rscq", name="rscq")
                nc.vector.reciprocal_approx_fast(out=rscq[:], in_=rsq[:])
                rinq = nrm_pool.tile([33, NQ], F32R, tag="rinq", name="rinq")
                nc.vector._custom_dve(
                    RECIPROCAL_APPROX_NR, out=rinq[:], in0=rsq[:], in1=rscq[:],
                    s0=2.0,
                )
                def bc_mul():
                    # single K=33 matmul: rows 1-31 of rinq are benign (1.0)
                    # and hit zero weights; two K=1 row-tiled matmuls would
                    # accumulate into the same psum region CONCURRENTLY
                    bc_ps = ps_w.tile([P, NQ], F32, tag="w", name="bc_ps")
                    nc.tensor.matmul(
                        bc_ps[:], lhsT=epat_sb[:], rhs=rinq[:],
                        start=True, stop=True,
                    )
                    nc.vector.tensor_mul(otn[:, qj, :], otb[:], bc_ps[:])
                return [bc_mul]

            def outproj_sc(b, sc, engine="alt"):
                qj, sc4 = sc // 4, sc % 4
                otn = st[b]["otn"]
                out_sb = osb_pool.tile([P, 2, NQ], F16, tag="out", name="out_sb")
                for fc in range(2):
                    ops = ps_w.tile([P, NQ], F32, tag="w", name="op_ps")
                    if with_bias_o:
                        nc.tensor.matmul(
                            ops[:], lhsT=ones16_sb[:],
                            rhs=bo8_sb[:, fc * NQ:(fc + 1) * NQ],
                            start=True, stop=False,
                        )
                    nc.tensor.matmul(
                        ops[:],
                        lhsT=otn[:, qj, sc4 * P:(sc4 + 1) * P],
                        rhs=wo_sb[:, fc * NQ:(fc + 1) * NQ],
                        start=not with_bias_o, stop=True,
                    )
                    if engine == "act":
                        nc.scalar.copy(out_sb[:, fc, :], ops[:])
                    else:
                        copy_dve(out_sb[:, fc, :], ops[:])
                nc.sync.dma_start(
                    out[b, sc * P:(sc + 1) * P, :],
                    out_sb.rearrange("p a n -> p (a n)"),
                )

            # ---------------- program ----------------
            alloc_qkv(0)
            alloc_qkv(1)

            # all proj/transpose work as packet blocks, pumped into the
            # attention stream with deadline barriers ((b, j) block tags).
            pre = collections.deque()
            post = collections.deque()  # deferred out-proj

            def add_block(b, j):
                for w_idx in range(3):
                    pre.append(((b, j),
                                lambda bb=b, w=w_idx, jj=j: proj_packet(bb, w, jj)))
                for sc in range(j * 4, (j + 1) * 4):
                    pre.append(((b, j), lambda bb=b, s=sc: v_transpose(bb, s)))

            # block (0,0) inline, d-major interleaved across q/k/v so the PE
            # tracks the xt half0 DMA chunk arrivals
            pq = ps_w.tile([P, NQ], F32, tag="w", name="pp_q0")
            pk = ps_w.tile([P, NQ], F32, tag="w", name="pp_k0")
            pv = ps_s.tile([P, 2, NQ], F32, tag="s", name="pp_v0")
            for d in range(DC):
                for pp, w_sb in ((pq, wq_sb), (pk, wk_sb), (pv, wv_sb)):
                    dst = pp[:, 0, :] if pp is pv else pp[:]
                    nc.tensor.matmul(
                        dst, lhsT=w_sb[:, d, :], rhs=xt_sbs[0][:, d, 0:NQ],
                        start=(d == 0), stop=(d == DC - 1),
                    )
            proj_copy(0, st[0]["qt"][:, 0, :], pq[:], "act")
            proj_copy(1, st[0]["kt"][:, 0, :], pk[:], "act")
            proj_copy(2, st[0]["vt"][:, 0, :], pv[:, 0, :], "act")
            for sc in range(4):
                v_transpose(0, sc)

            for j in range(1, QJ):
                add_block(0, j)
            for j in range(QJ):
                add_block(1, j)

            groups_left = [2 * B * sum(range(1, QJ + 1)) if causal
                           else B * (KC // GK)]
            _ptog = [0]

            def pump():
                groups_left[0] -= 1
                backlog = len(pre) + len(post)
                n = 2 if backlog > groups_left[0] else 1
                for _ in range(n):
                    _ptog[0] ^= 1
                    if pre and (_ptog[0] or not post):
                        pre.popleft()[1]()
                    elif post:
                        post.popleft()()

            def flush_pre(b, qj):
                while pre and pre[0][0][0] == b and pre[0][0][1] <= qj:
                    pre.popleft()[1]()

            attn_alloc(0)
            attn_alloc(1)
            sched = [(0, 0), (0, 1), (0, 2), (0, 3), (1, 0), (1, 1), (1, 2),
                     (1, 3)]
            for i, (b, qj) in enumerate(sched):
                flush_pre(b, qj)
                o_ps = attn_qj(b, qj, pump)
                tail = norm_qj(b, qj, o_ps, last=(i == len(sched) - 1))
                if i < len(sched) - 1:
                    for fn in tail:
                        post.append(fn)
                    for sc in range(qj * 4, (qj + 1) * 4):
                        post.append(lambda bb=b, s=sc: outproj_sc(bb, s))
                else:
                    for fn in tail:
                        fn()

            while pre:
                pre.popleft()[1]()
            while post:
                post.popleft()()
            for sc in range(12, 16):
                outproj_sc(1, sc, engine="act")

    nc.compile()
    _NC_CACHE[key] = nc
    return nc


def _check_causal(mask: np.ndarray) -> bool:
    m = np.asarray(mask).reshape(mask.shape[-2], mask.shape[-1])
    s = m.shape[0]
    if np.array_equal(m, np.tril(np.ones((s, s), dtype=bool))):
        return True
    if m.all():
        return False
    raise NotImplementedError("only causal or all-true masks are supported")


def _warr(w):
    # [D, HH] -> [P, DC, HH] fp16 (partition-major chunks, contiguous DMA)
    return np.ascontiguousarray(
        w.reshape(DC, P, HH).transpose(1, 0, 2).astype(np.float16))


def kernel(inputs_q, mask, Wq, bq, Wk, bk, Wv, bv, Wo, bo, _trace=False,
           _trace_cores=None):
    inputs_q = np.asarray(inputs_q, dtype=np.float32)
    Wq = np.asarray(Wq, dtype=np.float32).reshape(D, H * HD)
    Wk = np.asarray(Wk, dtype=np.float32).reshape(D, H * HD)
    Wv = np.asarray(Wv, dtype=np.float32).reshape(D, H * HD)
    Wo = np.asarray(Wo, dtype=np.float32).reshape(H * HD, D)
    bq = np.asarray(bq, dtype=np.float32).reshape(H * HD)
    bk = np.asarray(bk, dtype=np.float32).reshape(H * HD)
    bv = np.asarray(bv, dtype=np.float32).reshape(H * HD)
    bo = np.asarray(bo, dtype=np.float32).reshape(D)

    causal = _check_causal(mask)
    with_bias_qkv = bool(bq.any() or bk.any() or bv.any())
    with_bias_o = bool(bo.any())

    nc = _build_nc(with_bias_qkv, with_bias_o, causal)

    xt = np.ascontiguousarray(
        inputs_q.transpose(0, 2, 1).astype(np.float16))  # [B, D, S]
    in_maps = []
    for c in range(NCORES):
        f0, f1 = c * HH, (c + 1) * HH
        m = {
            "xt": xt,
            "wq": _warr(Wq[:, f0:f1]),
            "wk": _warr(Wk[:, f0:f1]),
            "wv": _warr(Wv[:, f0:f1]),
            "wo": np.ascontiguousarray(Wo[f0:f1, :].astype(np.float16)),
        }
        if with_bias_qkv:
            m["bqkv"] = np.ascontiguousarray(
                np.stack([bq[f0:f1], bk[f0:f1], bv[f0:f1]])
            )
        if with_bias_o:
            m["bo8"] = np.ascontiguousarray((bo / NCORES).astype(np.float16))
        in_maps.append(m)

    kwargs = {}
    if _trace:
        kwargs["trace"] = True
        if _trace_cores is not None:
            kwargs["trace_cores"] = _trace_cores
    res = run_bass_kernel_spmd(nc, in_maps, core_ids=list(range(NCORES)), **kwargs)

    acc = np.zeros((B, S, D), dtype=np.float32)
    for c in range(NCORES):
        acc += res.results[c]["out"].astype(np.float32)
    if not with_bias_o:
        acc += bo  # bo is zero here, but keep the math explicit
    out = acc.astype(np.float32)
    if _trace:
        return out, res
    return out


# revision 6
# speedup vs baseline: 1.0610x; 1.0261x over previous
"""Causal multi-head attention (B=2, S=2048, D=1024, H=16, HD=64) on 8 trn2 cores.

Sharding: 2 heads per core x both batches (head-parallel QKV/attention/out-proj,
Wo h-split => per-core partial outputs, summed on host).

All matmul operands are fp16 (half DMA/SBUF vs fp32, same 1 cyc/row PE rate,
DVE 2x modes); PSUM accumulation, softmax denominators and normalization stay
fp32. Verified on HW: rel err 6.8e-4 vs the fp32 reference.

Structure (single fused stream, engines balanced):
  - x^T is DMA'd in s-halves; the first q/k/v projection packets interleave
    d-major with the chunk arrivals so the PE tracks the DMA
  - all other projections are single-psum-bank packets pumped into the
    attention stream with deadline barriers ((b, j) block tags)
  - scores pair the two heads at partition bases 0/64 (row tiling, K=64);
    exp on ACT only; causal masking = DVE multiply with 2 static fp16 tiles
    (the affine iota pattern is qj-independent)
  - attnV runs lagged one group behind scores so the PE FIFO never
    head-of-line blocks on the exp
  - V^T via PE fp16 transposes through the shared work psum ring
  - normalization: den rows copied to SBUF (custom DVE recip ops must NOT
    read PSUM: partition-addressing corruption observed on HW), packed
    recip chain at partitions 0/32, two accumulating K=1 E-pattern matmuls
    broadcast both heads' reciprocals into one [128,NQ] psum tile, one
    whole-width multiply; broadcast+out-proj deferred into the pump stream
  - gpsimd only does f32 memset/affine_select (partition_broadcast and
    fp16 gpsimd ucode paths produce NaN on this HW)

PSUM (8 banks): ps_s = scores + first v packet [P,2,NQ]x2 (4), ps_o = attnV
accumulators [HD+1,NQ]x2 (2), ps_w = out-proj / proj packets / V-transposes /
bc broadcasts [P,NQ]x2 (2).
"""

import collections

import numpy as np

import concourse.bass as bass
import concourse.mybir as mybir
import concourse.tile as tile
from concourse import bacc
from concourse.bass_utils import run_bass_kernel_spmd
from concourse.masks import make_identity
from concourse.dve_ops import RECIPROCAL_APPROX_NR

F32 = mybir.dt.float32
F32R = mybir.dt.float32r
F16 = mybir.dt.float16
AF = mybir.ActivationFunctionType

B, S, D, H, HD = 2, 2048, 1024, 16, 64
NCORES = 8
HPC = H // NCORES          # heads per core = 2
HH = HPC * HD              # 128 concat head dims per core
P = 128
DC = D // P                # 8 d-chunks
NQ = 512                   # q tile (psum bank width fp32)
QJ = S // NQ               # 4 q tiles
KC = S // P                # 16 k chunks
GK = 2                     # k-chunks per score group (psum: [128, GK, NQ])

_NC_CACHE = {}


def _build_nc(with_bias_qkv: bool, with_bias_o: bool, causal: bool):
    key = (with_bias_qkv, with_bias_o, causal)
    if key in _NC_CACHE:
        return _NC_CACHE[key]

    nc = bacc.Bacc("TRN2", target_bir_lowering=False, debug=False)
    xt = nc.dram_tensor("xt", [B, D, S], F16, kind="ExternalInput")
    wq = nc.dram_tensor("wq", [P, DC, HH], F16, kind="ExternalInput")
    wk = nc.dram_tensor("wk", [P, DC, HH], F16, kind="ExternalInput")
    wv = nc.dram_tensor("wv", [P, DC, HH], F16, kind="ExternalInput")
    wo = nc.dram_tensor("wo", [HH, D], F16, kind="ExternalInput")
    if with_bias_qkv:
        bqkv = nc.dram_tensor("bqkv", [3, HH], F32, kind="ExternalInput")
    if with_bias_o:
        bo8 = nc.dram_tensor("bo8", [D], F16, kind="ExternalInput")
    out = nc.dram_tensor("out", [B, S, D], F16, kind="ExternalOutput")

    with tile.TileContext(nc) as tc:
        with (
            tc.tile_pool(name="const", bufs=1) as cpool,
            tc.tile_pool(name="xtp", bufs=2) as xt_pool,
            tc.tile_pool(name="qkv", bufs=2) as qkv_pool,
            tc.tile_pool(name="otn", bufs=2) as otn_pool,
            tc.tile_pool(name="nrm", bufs=4) as nrm_pool,
            tc.tile_pool(name="ep", bufs=10) as e_pool,
            tc.tile_pool(name="osb", bufs=3) as osb_pool,
            tc.tile_pool(name="ps_s", bufs=2, space="PSUM") as ps_s,
            tc.tile_pool(name="ps_o", bufs=2, space="PSUM") as ps_o,
            tc.tile_pool(name="ps_w", bufs=2, space="PSUM") as ps_w,
        ):
            # ---- constants / weights (DMA order matters: wq then xt b0) ----
            wq_sb = cpool.tile([P, DC, HH], F16, tag="wq", name="wq_sb")
            wk_sb = cpool.tile([P, DC, HH], F16, tag="wk", name="wk_sb")
            wv_sb = cpool.tile([P, DC, HH], F16, tag="wv", name="wv_sb")
            wo_sb = cpool.tile([HH, D], F16, tag="wo", name="wo_sb")
            nc.sync.dma_start(wq_sb[:], wq[:])

            xt_sbs = []

            def alloc_xt(b):
                xt_sb = xt_pool.tile([P, DC, S], F16, tag="xt", name="xt_sb")
                xt_sbs.append(xt_sb)

            def load_xt_half(b, half):
                s0, s1 = half * (S // 2), (half + 1) * (S // 2)
                for d in range(DC):
                    nc.sync.dma_start(xt_sbs[b][:, d, s0:s1],
                                      xt[b, d * P:(d + 1) * P, s0:s1])

            alloc_xt(0)
            alloc_xt(1)
            nc.sync.dma_start(wk_sb[:], wk[:])
            load_xt_half(0, 0)
            nc.sync.dma_start(wv_sb[:], wv[:])
            load_xt_half(0, 1)
            load_xt_half(1, 0)
            load_xt_half(1, 1)
            nc.sync.dma_start(wo_sb[:], wo[:])
            if with_bias_qkv:
                bqkvt_sb = cpool.tile([HH, 3], F32, tag="bqkvt", name="bqkvt_sb")
                for i in range(3):
                    nc.sync.dma_start(
                        bqkvt_sb[:, i:i + 1], bqkv[i:i + 1, :].rearrange("a f -> f a")
                    )
            if with_bias_o:
                bo8_sb = cpool.tile([1, D], F16, tag="bo8", name="bo8_sb")
                nc.sync.dma_start(bo8_sb[:], bo8.rearrange("(a d) -> a d", a=1))
                ones16_sb = cpool.tile([1, P], F16, tag="ones16", name="ones16_sb")
                nc.vector.memset(ones16_sb[:], 1.0)

            ident32_sb = cpool.tile([P, P], F32, tag="ident32", name="ident32_sb")
            make_identity(nc, ident32_sb[:])
            ident_sb = cpool.tile([P, P], F16, tag="ident", name="ident_sb")
            nc.vector.tensor_copy(ident_sb[:], ident32_sb[:])

            # E-pattern for the denominator broadcast matmuls (f32r bits ==
            # f32): row at partition 32h covers output partitions 64h..64h+63,
            # so two accumulating K=1 matmuls broadcast both heads'
            # reciprocals into one [128, NQ] psum tile
            epat_sb = cpool.tile([33, P], F32R, tag="epat", name="epat_sb")
            nc.vector.memset(epat_sb[:].bitcast(F32), 0.0)
            nc.vector.memset(epat_sb[0:1, 0:HD].bitcast(F32), 1.0)
            nc.vector.memset(epat_sb[32:33, HD:P].bitcast(F32), 1.0)

            # causal mask tiles: iota(p, c2, col) = base - P*c2 + col - p >= 0
            # keeps q >= k. For diagonal group g in {2qj, 2qj+1} the base is
            # qj*NQ - g*GK*P = 0 (even g) or -2*P (odd g): qj-independent.
            # Generated in f32 (gpsimd ucode path proven for 32-bit only),
            # then converted to f16 on DVE.
            masks = []
            if causal:
                m32 = cpool.tile([P, GK, NQ], F32, tag="mask32", name="mask32_sb")
                for i in range(2):
                    nc.vector.memset(m32[:], 1.0)
                    nc.gpsimd.affine_select(
                        out=m32[:], in_=m32[:],
                        compare_op=mybir.AluOpType.is_ge, fill=0.0,
                        base=-2 * P * i,
                        pattern=[[-P, GK], [1, NQ]],
                        channel_multiplier=-1,
                    )
                    m = cpool.tile([P, GK, NQ], F16, tag=f"mask{i}",
                                   name=f"mask{i}_sb")
                    nc.vector.tensor_copy(m[:], m32[:])
                    masks.append(m)

            # per-batch state
            st = [dict() for _ in range(B)]

            # alternating engine for psum->sbuf copies
            _alt = [0]

            def copy_alt(dst, src):
                _alt[0] ^= 1
                if _alt[0]:
                    nc.scalar.copy(dst, src)
                else:
                    nc.vector.tensor_copy(dst, src)

            def copy_dve(dst, src):
                nc.vector.tensor_copy(dst, src)

            def proj_copy(w_idx, dst_ap, pps_ap, engine):
                if with_bias_qkv:
                    nc.scalar.activation(
                        dst_ap, pps_ap, AF.Identity,
                        bias=bqkvt_sb[:, w_idx:w_idx + 1],
                    )
                elif engine == "act":
                    nc.scalar.copy(dst_ap, pps_ap)
                elif engine == "dve":
                    nc.vector.tensor_copy(dst_ap, pps_ap)
                else:
                    copy_alt(dst_ap, pps_ap)

            def alloc_qkv(b):
                st[b]["qt"] = qkv_pool.tile([P, QJ, NQ], F16, tag="qt", name="qt_sb")
                st[b]["kt"] = qkv_pool.tile([P, QJ, NQ], F16, tag="kt", name="kt_sb")
                st[b]["vt"] = qkv_pool.tile([P, QJ, NQ], F16, tag="vt", name="vt_sb")
                v_sb = qkv_pool.tile([P, KC, HPC, HD + 1], F16, tag="v", name="v_sb")
                nc.vector.memset(v_sb[:, :, :, HD:], 1.0)
                st[b]["v"] = v_sb

            def proj_packet(b, w_idx, j, engine="alt"):
                # single-bank d-inner packet through the ps_w ring
                w_sb = (wq_sb, wk_sb, wv_sb)[w_idx]
                dst = (st[b]["qt"], st[b]["kt"], st[b]["vt"])[w_idx]
                pp = ps_w.tile([P, NQ], F32, tag="w", name="proj_pp")
                for d in range(DC):
                    nc.tensor.matmul(
                        pp[:],
                        lhsT=w_sb[:, d, :],
                        rhs=xt_sbs[b][:, d, j * NQ:(j + 1) * NQ],
                        start=(d == 0), stop=(d == DC - 1),
                    )
                proj_copy(w_idx, dst[:, j, :], pp[:], engine)

            def v_transpose(b, sc):
                tp = ps_w.tile([P, P], F16, tag="w", name="tr_ps")
                nc.tensor.transpose(
                    tp[:], st[b]["vt"][:, sc // 4, (sc % 4) * P:(sc % 4 + 1) * P],
                    ident_sb[:],
                )
                # single strided copy: tp[:, h*64+d] -> v[:, sc, h, d]
                nc.scalar.copy(
                    st[b]["v"][:, sc, :, :HD],
                    tp.rearrange("p (h d) -> p h d", h=HPC),
                )

            def attn_alloc(b):
                st[b]["otn"] = otn_pool.tile([P, QJ, NQ], F16, tag="otn",
                                             name="otn_sb")

            def attn_qj(b, qj, pump):
                """scores/exp for group g; attnV lagged one group."""
                qt, kt, v = st[b]["qt"], st[b]["kt"], st[b]["v"]
                ngroups = (2 * (qj + 1)) if causal else (KC // GK)
                o_ps = [ps_o.tile([HD + 1, NQ], F32, tag="o", name=f"o_ps{h}")
                        for h in range(HPC)]

                def attn_v(g, e_tiles):
                    for h in range(HPC):
                        for c2 in range(GK):
                            ki = g * GK + c2
                            nc.tensor.matmul(
                                o_ps[h][:],
                                lhsT=v[:, ki, h, :],
                                rhs=e_tiles[h][:, c2, :],
                                start=(g == 0 and c2 == 0),
                                stop=(g == ngroups - 1 and c2 == GK - 1),
                            )

                pend = []
                for g in range(ngroups):
                    stp = []
                    for h in range(HPC):
                        h0 = h * HD
                        sp = ps_s.tile([P, GK, NQ], F32, tag="s", name=f"st_ps{h}")
                        stp.append(sp)
                        for c2 in range(GK):
                            ki = g * GK + c2
                            nc.tensor.matmul(
                                sp[:, c2, :],
                                lhsT=kt[h0:h0 + HD, ki // 4, (ki % 4) * P:(ki % 4 + 1) * P],
                                rhs=qt[h0:h0 + HD, qj, :],
                                start=True, stop=True,
                            )
                    e_tiles = []
                    for h in range(HPC):
                        e_sb = e_pool.tile([P, GK, NQ], F16, tag="e", name="e_sb")
                        nc.scalar.activation(e_sb[:], stp[h][:], AF.Exp, scale=0.125)
                        if causal and g >= 2 * qj:
                            nc.vector.tensor_mul(e_sb[:], e_sb[:],
                                                 masks[g - 2 * qj][:])
                        e_tiles.append(e_sb)
                    pend.append((g, e_tiles))
                    if len(pend) > 3:
                        attn_v(*pend.pop(0))
                    pump()
                for it in pend:
                    attn_v(*it)
                return o_ps

            def norm_qj(b, qj, o_ps, last=False):
                # den rows copied to SBUF first (custom DVE recip ops must NOT
                # read PSUM: observed partition-addressing corruption on HW),
                # both heads packed at partitions 0/1, one recip chain, then
                # one E-pattern broadcast matmul + one whole-width scale
                otn = st[b]["otn"]
                rsq = nrm_pool.tile([33, NQ], F32, tag="rsq", name="rsq")
                # partitions 1-31 are never consumed (epat matmuls read only
                # rows 0/32) but the packed recip ops scan them: give the tile
                # a benign fill on the idle Pool engine before the den copies
                nc.gpsimd.memset(rsq[:, :], 1.0)
                otb = nrm_pool.tile([P, NQ], F32, tag="ot", name="otb")
                for h in range(HPC):
                    # den-row + o copies first: each head's psum accumulator
                    # releases after two DVE ops, before the recip chain
                    nc.vector.tensor_copy(
                        rsq[32 * h:32 * h + 1, :], o_ps[h][HD:HD + 1, :])
                    if last:
                        # tail: ACT is exp-idle; keep DVE free for the recips
                        nc.scalar.copy(
                            otb[h * HD:(h + 1) * HD, :], o_ps[h][:HD, :])
                    else:
                        nc.vector.tensor_copy(
                            otb[h * HD:(h + 1) * HD, :], o_ps[h][:HD, :])
                rscq = nrm_pool.tile([33, NQ], F32, tag="rscq", name="rscq")
                nc.vector.reciprocal_approx_fast(out=rscq[:], in_=rsq[:])
                rinq = nrm_pool.tile([33, NQ], F32R, tag="rinq", name="rinq")
                nc.vector._custom_dve(
                    RECIPROCAL_APPROX_NR, out=rinq[:], in0=rsq[:], in1=rscq[:],
                    s0=2.0,
                )
                def bc_mul():
                    # single K=33 matmul: rows 1-31 of rinq are benign (1.0)
                    # and hit zero weights; two K=1 row-tiled matmuls would
                    # accumulate into the same psum region CONCURRENTLY
                    bc_ps = ps_w.tile([P, NQ], F32, tag="w", name="bc_ps")
                    nc.tensor.matmul(
                        bc_ps[:], lhsT=epat_sb[:], rhs=rinq[:],
                        start=True, stop=True,
                    )
                    nc.vector.tensor_mul(otn[:, qj, :], otb[:], bc_ps[:])
                return [bc_mul]

            def outproj_sc(b, sc, engine="alt"):
                qj, sc4 = sc // 4, sc % 4
                otn = st[b]["otn"]
                out_sb = osb_pool.tile([P, 2, NQ], F16, tag="out", name="out_sb")
                for fc in range(2):
                    ops = ps_w.tile([P, NQ], F32, tag="w", name="op_ps")
                    if with_bias_o:
                        nc.tensor.matmul(
                            ops[:], lhsT=ones16_sb[:],
                            rhs=bo8_sb[:, fc * NQ:(fc + 1) * NQ],
                            start=True, stop=False,
                        )
                    nc.tensor.matmul(
                        ops[:],
                        lhsT=otn[:, qj, sc4 * P:(sc4 + 1) * P],
                        rhs=wo_sb[:, fc * NQ:(fc + 1) * NQ],
                        start=not with_bias_o, stop=True,
                    )
                    if engine == "act":
                        nc.scalar.copy(out_sb[:, fc, :], ops[:])
                    else:
                        copy_dve(out_sb[:, fc, :], ops[:])
                nc.sync.dma_start(
                    out[b, sc * P:(sc + 1) * P, :],
                    out_sb.rearrange("p a n -> p (a n)"),
                )

            # ---------------- program ----------------
            alloc_qkv(0)
            alloc_qkv(1)

            # all proj/transpose work as packet blocks, pumped into the
            # attention stream with deadline barriers ((b, j) block tags).
            pre = collections.deque()
            post = collections.deque()  # deferred out-proj

            def add_block(b, j):
                for w_idx in range(3):
                    pre.append(((b, j),
                                lambda bb=b, w=w_idx, jj=j: proj_packet(bb, w, jj)))
                for sc in range(j * 4, (j + 1) * 4):
                    pre.append(((b, j), lambda bb=b, s=sc: v_transpose(bb, s)))

            # block (0,0) inline, d-major interleaved across q/k/v so the PE
            # tracks the xt half0 DMA chunk arrivals
            pq = ps_w.tile([P, NQ], F32, tag="w", name="pp_q0")
            pk = ps_w.tile([P, NQ], F32, tag="w", name="pp_k0")
            pv = ps_s.tile([P, 2, NQ], F32, tag="s", name="pp_v0")
            for d in range(DC):
                for pp, w_sb in ((pq, wq_sb), (pk, wk_sb), (pv, wv_sb)):
                    dst = pp[:, 0, :] if pp is pv else pp[:]
                    nc.tensor.matmul(
                        dst, lhsT=w_sb[:, d, :], rhs=xt_sbs[0][:, d, 0:NQ],
                        start=(d == 0), stop=(d == DC - 1),
                    )
            proj_copy(0, st[0]["qt"][:, 0, :], pq[:], "act")
            proj_copy(1, st[0]["kt"][:, 0, :], pk[:], "act")
            proj_copy(2, st[0]["vt"][:, 0, :], pv[:, 0, :], "act")
            for sc in range(4):
                v_transpose(0, sc)

            for j in range(1, QJ):
                add_block(0, j)
            for j in range(QJ):
                add_block(1, j)

            groups_left = [2 * B * sum(range(1, QJ + 1)) if causal
                           else B * (KC // GK)]
            _ptog = [0]

            def pump():
                groups_left[0] -= 1
                backlog = len(pre) + len(post)
                n = 2 if backlog > groups_left[0] else 1
                for _ in range(n):
                    _ptog[0] ^= 1
                    if pre and (_ptog[0] or not post):
                        pre.popleft()[1]()
                    elif post:
                        post.popleft()()

            def flush_pre(b, qj):
                while pre and pre[0][0][0] == b and pre[0][0][1] <= qj:
                    pre.popleft()[1]()

            attn_alloc(0)
            attn_alloc(1)
            sched = [(0, 0), (0, 1), (0, 2), (0, 3), (1, 0), (1, 1), (1, 2),
                     (1, 3)]
            for i, (b, qj) in enumerate(sched):
                flush_pre(b, qj)
                o_ps = attn_qj(b, qj, pump)
                tail = norm_qj(b, qj, o_ps, last=(i == len(sched) - 1))
                if i < len(sched) - 1:
                    for fn in tail:
                        post.append(fn)
                    for sc in range(qj * 4, (qj + 1) * 4):
                        post.append(lambda bb=b, s=sc: outproj_sc(bb, s))
                else:
                    for fn in tail:
                        fn()

            while pre:
                pre.popleft()[1]()
            while post:
                post.popleft()()
            for sc in range(12, 16):
                outproj_sc(1, sc, engine="act")

    nc.compile()
    _NC_CACHE[key] = nc
    return nc


def _check_causal(mask: np.ndarray) -> bool:
    m = np.asarray(mask).reshape(mask.shape[-2], mask.shape[-1])
    s = m.shape[0]
    if np.array_equal(m, np.tril(np.ones((s, s), dtype=bool))):
        return True
    if m.all():
        return False
    raise NotImplementedError("only causal or all-true masks are supported")


def _warr(w):
    # [D, HH] -> [P, DC, HH] fp16 (partition-major chunks, contiguous DMA)
    return np.ascontiguousarray(
        w.reshape(DC, P, HH).transpose(1, 0, 2).astype(np.float16))


def kernel(inputs_q, mask, Wq, bq, Wk, bk, Wv, bv, Wo, bo, _trace=False,
           _trace_cores=None):
    inputs_q = np.asarray(inputs_q, dtype=np.float32)
    Wq = np.asarray(Wq, dtype=np.float32).reshape(D, H * HD)
    Wk = np.asarray(Wk, dtype=np.float32).reshape(D, H * HD)
    Wv = np.asarray(Wv, dtype=np.float32).reshape(D, H * HD)
    Wo = np.asarray(Wo, dtype=np.float32).reshape(H * HD, D)
    bq = np.asarray(bq, dtype=np.float32).reshape(H * HD)
    bk = np.asarray(bk, dtype=np.float32).reshape(H * HD)
    bv = np.asarray(bv, dtype=np.float32).reshape(H * HD)
    bo = np.asarray(bo, dtype=np.float32).reshape(D)

    causal = _check_causal(mask)
    with_bias_qkv = bool(bq.any() or bk.any() or bv.any())
    with_bias_o = bool(bo.any())

    nc = _build_nc(with_bias_qkv, with_bias_o, causal)

    xt = np.ascontiguousarray(
        inputs_q.transpose(0, 2, 1).astype(np.float16))  # [B, D, S]
    in_maps = []
    for c in range(NCORES):
        f0, f1 = c * HH, (c + 1) * HH
        m = {
            "xt": xt,
            "wq": _warr(Wq[:, f0:f1]),
            "wk": _warr(Wk[:, f0:f1]),
            "wv": _warr(Wv[:, f0:f1]),
            "wo": np.ascontiguousarray(Wo[f0:f1, :].astype(np.float16)),
        }
        if with_bias_qkv:
            m["bqkv"] = np.ascontiguousarray(
                np.stack([bq[f0:f1], bk[f0:f1], bv[f0:f1]])
            )
        if with_bias_o:
            m["bo8"] = np.ascontiguousarray((bo / NCORES).astype(np.float16))
        in_maps.append(m)

    kwargs = {}
    if _trace:
        kwargs["trace"] = True
        if _trace_cores is not None:
            kwargs["trace_cores"] = _trace_cores
    res = run_bass_kernel_spmd(nc, in_maps, core_ids=list(range(NCORES)), **kwargs)

    acc = np.zeros((B, S, D), dtype=np.float32)
    for c in range(NCORES):
        acc += res.results[c]["out"].astype(np.float32)
    if not with_bias_o:
        acc += bo  # bo is zero here, but keep the math explicit
    out = acc.astype(np.float32)
    if _trace:
        return out, res
    return out


# revision 7
# speedup vs baseline: 1.0695x; 1.0080x over previous
"""Causal multi-head attention (B=2, S=2048, D=1024, H=16, HD=64) on 8 trn2 cores.

Sharding: 2 heads per core x both batches (head-parallel QKV/attention/out-proj,
Wo h-split => per-core partial outputs, summed on host).

All matmul operands are fp16 (half DMA/SBUF vs fp32, same 1 cyc/row PE rate,
DVE 2x modes); PSUM accumulation, softmax denominators and normalization stay
fp32. Verified on HW: rel err 6.8e-4 vs the fp32 reference.

Structure (single fused stream, engines balanced):
  - x^T is DMA'd in s-halves; the first q/k/v projection packets interleave
    d-major with the chunk arrivals so the PE tracks the DMA
  - all other projections are single-psum-bank packets pumped into the
    attention stream with deadline barriers ((b, j) block tags)
  - scores pair the two heads at partition bases 0/64 (row tiling, K=64);
    exp on ACT only; causal masking = DVE multiply with 2 static fp16 tiles
    (the affine iota pattern is qj-independent)
  - attnV runs lagged one group behind scores so the PE FIFO never
    head-of-line blocks on the exp
  - V^T via PE fp16 transposes through the shared work psum ring
  - normalization: den rows copied to SBUF (custom DVE recip ops must NOT
    read PSUM: partition-addressing corruption observed on HW), packed
    recip chain at partitions 0/32, two accumulating K=1 E-pattern matmuls
    broadcast both heads' reciprocals into one [128,NQ] psum tile, one
    whole-width multiply; broadcast+out-proj deferred into the pump stream
  - gpsimd only does f32 memset/affine_select (partition_broadcast and
    fp16 gpsimd ucode paths produce NaN on this HW)

PSUM (8 banks): ps_s = scores + first v packet [P,2,NQ]x2 (4), ps_o = attnV
accumulators [HD+1,NQ]x2 (2), ps_w = out-proj / proj packets / V-transposes /
bc broadcasts [P,NQ]x2 (2).
"""

import collections

import numpy as np

import concourse.bass as bass
import concourse.mybir as mybir
import concourse.tile as tile
from concourse import bacc
from concourse.bass_utils import run_bass_kernel_spmd
from concourse.masks import make_identity
from concourse.dve_ops import RECIPROCAL_APPROX_NR

F32 = mybir.dt.float32
F32R = mybir.dt.float32r
F16 = mybir.dt.float16
AF = mybir.ActivationFunctionType

B, S, D, H, HD = 2, 2048, 1024, 16, 64
NCORES = 8
HPC = H // NCORES          # heads per core = 2
HH = HPC * HD              # 128 concat head dims per core
P = 128
DC = D // P                # 8 d-chunks
NQ = 512                   # q tile (psum bank width fp32)
QJ = S // NQ               # 4 q tiles
KC = S // P                # 16 k chunks
GK = 2                     # k-chunks per score group (psum: [128, GK, NQ])

_NC_CACHE = {}


def _build_nc(with_bias_qkv: bool, with_bias_o: bool, causal: bool):
    key = (with_bias_qkv, with_bias_o, causal)
    if key in _NC_CACHE:
        return _NC_CACHE[key]

    nc = bacc.Bacc("TRN2", target_bir_lowering=False, debug=False)
    xt = nc.dram_tensor("xt", [B, D, S], F16, kind="ExternalInput")
    wq = nc.dram_tensor("wq", [P, DC, HH], F16, kind="ExternalInput")
    wk = nc.dram_tensor("wk", [P, DC, HH], F16, kind="ExternalInput")
    wv = nc.dram_tensor("wv", [P, DC, HH], F16, kind="ExternalInput")
    wo = nc.dram_tensor("wo", [HH, D], F16, kind="ExternalInput")
    if with_bias_qkv:
        bqkv = nc.dram_tensor("bqkv", [3, HH], F32, kind="ExternalInput")
    if with_bias_o:
        bo8 = nc.dram_tensor("bo8", [D], F16, kind="ExternalInput")
    out = nc.dram_tensor("out", [B, S, D], F16, kind="ExternalOutput")

    with tile.TileContext(nc) as tc:
        with (
            tc.tile_pool(name="const", bufs=1) as cpool,
            tc.tile_pool(name="xtp", bufs=2) as xt_pool,
            tc.tile_pool(name="qkv", bufs=2) as qkv_pool,
            tc.tile_pool(name="otn", bufs=2) as otn_pool,
            tc.tile_pool(name="nrm", bufs=4) as nrm_pool,
            tc.tile_pool(name="ep", bufs=10) as e_pool,
            tc.tile_pool(name="osb", bufs=4) as osb_pool,
            tc.tile_pool(name="ps_s", bufs=2, space="PSUM") as ps_s,
            tc.tile_pool(name="ps_o", bufs=2, space="PSUM") as ps_o,
            tc.tile_pool(name="ps_w", bufs=2, space="PSUM") as ps_w,
        ):
            # ---- constants / weights (DMA order matters: wq then xt b0) ----
            wq_sb = cpool.tile([P, DC, HH], F16, tag="wq", name="wq_sb")
            wk_sb = cpool.tile([P, DC, HH], F16, tag="wk", name="wk_sb")
            wv_sb = cpool.tile([P, DC, HH], F16, tag="wv", name="wv_sb")
            wo_sb = cpool.tile([HH, D], F16, tag="wo", name="wo_sb")
            nc.sync.dma_start(wq_sb[:], wq[:])

            xt_sbs = []

            def alloc_xt(b):
                xt_sb = xt_pool.tile([P, DC, S], F16, tag="xt", name="xt_sb")
                xt_sbs.append(xt_sb)

            def load_xt_half(b, half):
                s0, s1 = half * (S // 2), (half + 1) * (S // 2)
                for d in range(DC):
                    nc.sync.dma_start(xt_sbs[b][:, d, s0:s1],
                                      xt[b, d * P:(d + 1) * P, s0:s1])

            alloc_xt(0)
            alloc_xt(1)
            nc.sync.dma_start(wk_sb[:], wk[:])
            load_xt_half(0, 0)
            nc.sync.dma_start(wv_sb[:], wv[:])
            load_xt_half(0, 1)
            load_xt_half(1, 0)
            load_xt_half(1, 1)
            nc.sync.dma_start(wo_sb[:], wo[:])
            if with_bias_qkv:
                bqkvt_sb = cpool.tile([HH, 3], F32, tag="bqkvt", name="bqkvt_sb")
                for i in range(3):
                    nc.sync.dma_start(
                        bqkvt_sb[:, i:i + 1], bqkv[i:i + 1, :].rearrange("a f -> f a")
                    )
            if with_bias_o:
                bo8_sb = cpool.tile([1, D], F16, tag="bo8", name="bo8_sb")
                nc.sync.dma_start(bo8_sb[:], bo8.rearrange("(a d) -> a d", a=1))
                ones16_sb = cpool.tile([1, P], F16, tag="ones16", name="ones16_sb")
                nc.vector.memset(ones16_sb[:], 1.0)

            ident32_sb = cpool.tile([P, P], F32, tag="ident32", name="ident32_sb")
            make_identity(nc, ident32_sb[:])
            ident_sb = cpool.tile([P, P], F16, tag="ident", name="ident_sb")
            nc.vector.tensor_copy(ident_sb[:], ident32_sb[:])

            # E-pattern for the denominator broadcast matmuls (f32r bits ==
            # f32): row at partition 32h covers output partitions 64h..64h+63,
            # so two accumulating K=1 matmuls broadcast both heads'
            # reciprocals into one [128, NQ] psum tile
            epat_sb = cpool.tile([33, P], F32R, tag="epat", name="epat_sb")
            nc.vector.memset(epat_sb[:].bitcast(F32), 0.0)
            nc.vector.memset(epat_sb[0:1, 0:HD].bitcast(F32), 1.0)
            nc.vector.memset(epat_sb[32:33, HD:P].bitcast(F32), 1.0)

            # causal mask tiles: iota(p, c2, col) = base - P*c2 + col - p >= 0
            # keeps q >= k. For diagonal group g in {2qj, 2qj+1} the base is
            # qj*NQ - g*GK*P = 0 (even g) or -2*P (odd g): qj-independent.
            # Generated in f32 (gpsimd ucode path proven for 32-bit only),
            # then converted to f16 on DVE.
            masks = []
            if causal:
                m32 = cpool.tile([P, GK, NQ], F32, tag="mask32", name="mask32_sb")
                for i in range(2):
                    nc.vector.memset(m32[:], 1.0)
                    nc.gpsimd.affine_select(
                        out=m32[:], in_=m32[:],
                        compare_op=mybir.AluOpType.is_ge, fill=0.0,
                        base=-2 * P * i,
                        pattern=[[-P, GK], [1, NQ]],
                        channel_multiplier=-1,
                    )
                    m = cpool.tile([P, GK, NQ], F16, tag=f"mask{i}",
                                   name=f"mask{i}_sb")
                    nc.vector.tensor_copy(m[:], m32[:])
                    masks.append(m)

            # per-batch state
            st = [dict() for _ in range(B)]

            # alternating engine for psum->sbuf copies
            _alt = [0]

            def copy_alt(dst, src):
                _alt[0] ^= 1
                if _alt[0]:
                    nc.scalar.copy(dst, src)
                else:
                    nc.vector.tensor_copy(dst, src)

            def copy_dve(dst, src):
                nc.vector.tensor_copy(dst, src)

            def proj_copy(w_idx, dst_ap, pps_ap, engine):
                if with_bias_qkv:
                    nc.scalar.activation(
                        dst_ap, pps_ap, AF.Identity,
                        bias=bqkvt_sb[:, w_idx:w_idx + 1],
                    )
                elif engine == "act":
                    nc.scalar.copy(dst_ap, pps_ap)
                elif engine == "dve":
                    nc.vector.tensor_copy(dst_ap, pps_ap)
                else:
                    copy_alt(dst_ap, pps_ap)

            def alloc_qkv(b):
                st[b]["qt"] = qkv_pool.tile([P, QJ, NQ], F16, tag="qt", name="qt_sb")
                st[b]["kt"] = qkv_pool.tile([P, QJ, NQ], F16, tag="kt", name="kt_sb")
                st[b]["vt"] = qkv_pool.tile([P, QJ, NQ], F16, tag="vt", name="vt_sb")
                v_sb = qkv_pool.tile([P, KC, HPC, HD + 1], F16, tag="v", name="v_sb")
                nc.vector.memset(v_sb[:, :, :, HD:], 1.0)
                st[b]["v"] = v_sb

            def proj_packet(b, w_idx, j, engine="alt"):
                # single-bank d-inner packet through the ps_w ring
                w_sb = (wq_sb, wk_sb, wv_sb)[w_idx]
                dst = (st[b]["qt"], st[b]["kt"], st[b]["vt"])[w_idx]
                pp = ps_w.tile([P, NQ], F32, tag="w", name="proj_pp")
                for d in range(DC):
                    nc.tensor.matmul(
                        pp[:],
                        lhsT=w_sb[:, d, :],
                        rhs=xt_sbs[b][:, d, j * NQ:(j + 1) * NQ],
                        start=(d == 0), stop=(d == DC - 1),
                    )
                proj_copy(w_idx, dst[:, j, :], pp[:], engine)

            def v_transpose(b, sc):
                tp = ps_w.tile([P, P], F16, tag="w", name="tr_ps")
                nc.tensor.transpose(
                    tp[:], st[b]["vt"][:, sc // 4, (sc % 4) * P:(sc % 4 + 1) * P],
                    ident_sb[:],
                )
                # single strided copy: tp[:, h*64+d] -> v[:, sc, h, d]
                nc.scalar.copy(
                    st[b]["v"][:, sc, :, :HD],
                    tp.rearrange("p (h d) -> p h d", h=HPC),
                )

            def attn_alloc(b):
                st[b]["otn"] = otn_pool.tile([P, QJ, NQ], F16, tag="otn",
                                             name="otn_sb")

            def attn_qj(b, qj, pump):
                """scores/exp for group g; attnV lagged one group."""
                qt, kt, v = st[b]["qt"], st[b]["kt"], st[b]["v"]
                ngroups = (2 * (qj + 1)) if causal else (KC // GK)
                o_ps = [ps_o.tile([HD + 1, NQ], F32, tag="o", name=f"o_ps{h}")
                        for h in range(HPC)]

                def attn_v(g, e_tiles):
                    for h in range(HPC):
                        for c2 in range(GK):
                            ki = g * GK + c2
                            nc.tensor.matmul(
                                o_ps[h][:],
                                lhsT=v[:, ki, h, :],
                                rhs=e_tiles[h][:, c2, :],
                                start=(g == 0 and c2 == 0),
                                stop=(g == ngroups - 1 and c2 == GK - 1),
                            )

                pend = []
                for g in range(ngroups):
                    stp = []
                    for h in range(HPC):
                        h0 = h * HD
                        sp = ps_s.tile([P, GK, NQ], F32, tag="s", name=f"st_ps{h}")
                        stp.append(sp)
                        for c2 in range(GK):
                            ki = g * GK + c2
                            nc.tensor.matmul(
                                sp[:, c2, :],
                                lhsT=kt[h0:h0 + HD, ki // 4, (ki % 4) * P:(ki % 4 + 1) * P],
                                rhs=qt[h0:h0 + HD, qj, :],
                                start=True, stop=True,
                            )
                    e_tiles = []
                    for h in range(HPC):
                        e_sb = e_pool.tile([P, GK, NQ], F16, tag="e", name="e_sb")
                        nc.scalar.activation(e_sb[:], stp[h][:], AF.Exp, scale=0.125)
                        if causal and g >= 2 * qj:
                            nc.vector.tensor_mul(e_sb[:], e_sb[:],
                                                 masks[g - 2 * qj][:])
                        e_tiles.append(e_sb)
                    pend.append((g, e_tiles))
                    if len(pend) > 3:
                        attn_v(*pend.pop(0))
                    pump()
                for it in pend:
                    attn_v(*it)
                return o_ps

            def norm_qj(b, qj, o_ps, last=False):
                # den rows copied to SBUF first (custom DVE recip ops must NOT
                # read PSUM: observed partition-addressing corruption on HW),
                # both heads packed at partitions 0/1, one recip chain, then
                # one E-pattern broadcast matmul + one whole-width scale
                otn = st[b]["otn"]
                rsq = nrm_pool.tile([33, NQ], F32, tag="rsq", name="rsq")
                # partitions 1-31 are never consumed (epat matmuls read only
                # rows 0/32) but the packed recip ops scan them: give the tile
                # a benign fill on the idle Pool engine before the den copies
                nc.gpsimd.memset(rsq[:, :], 1.0)
                otb = nrm_pool.tile([P, NQ], F32, tag="ot", name="otb")
                for h in range(HPC):
                    # den-row + o copies first: each head's psum accumulator
                    # releases after two DVE ops, before the recip chain
                    nc.vector.tensor_copy(
                        rsq[32 * h:32 * h + 1, :], o_ps[h][HD:HD + 1, :])
                    if last:
                        # tail: ACT is exp-idle; keep DVE free for the recips
                        nc.scalar.copy(
                            otb[h * HD:(h + 1) * HD, :], o_ps[h][:HD, :])
                    else:
                        nc.vector.tensor_copy(
                            otb[h * HD:(h + 1) * HD, :], o_ps[h][:HD, :])
                rscq = nrm_pool.tile([33, NQ], F32, tag="rscq", name="rscq")
                nc.vector.reciprocal_approx_fast(out=rscq[:], in_=rsq[:])
                rinq = nrm_pool.tile([33, NQ], F32R, tag="rinq", name="rinq")
                nc.vector._custom_dve(
                    RECIPROCAL_APPROX_NR, out=rinq[:], in0=rsq[:], in1=rscq[:],
                    s0=2.0,
                )
                def bc_mul():
                    # single K=33 matmul: rows 1-31 of rinq are benign (1.0)
                    # and hit zero weights; two K=1 row-tiled matmuls would
                    # accumulate into the same psum region CONCURRENTLY
                    bc_ps = ps_w.tile([P, NQ], F32, tag="w", name="bc_ps")
                    nc.tensor.matmul(
                        bc_ps[:], lhsT=epat_sb[:], rhs=rinq[:],
                        start=True, stop=True,
                    )
                    nc.vector.tensor_mul(otn[:, qj, :], otb[:], bc_ps[:])
                return [bc_mul]

            def outproj_sc(b, sc, engine="alt"):
                qj, sc4 = sc // 4, sc % 4
                otn = st[b]["otn"]
                out_sb = osb_pool.tile([P, 2, NQ], F16, tag="out", name="out_sb")
                for fc in range(2):
                    ops = ps_w.tile([P, NQ], F32, tag="w", name="op_ps")
                    if with_bias_o:
                        nc.tensor.matmul(
                            ops[:], lhsT=ones16_sb[:],
                            rhs=bo8_sb[:, fc * NQ:(fc + 1) * NQ],
                            start=True, stop=False,
                        )
                    nc.tensor.matmul(
                        ops[:],
                        lhsT=otn[:, qj, sc4 * P:(sc4 + 1) * P],
                        rhs=wo_sb[:, fc * NQ:(fc + 1) * NQ],
                        start=not with_bias_o, stop=True,
                    )
                    if engine == "act":
                        nc.scalar.copy(out_sb[:, fc, :], ops[:])
                    else:
                        copy_dve(out_sb[:, fc, :], ops[:])
                nc.sync.dma_start(
                    out[b, sc * P:(sc + 1) * P, :],
                    out_sb.rearrange("p a n -> p (a n)"),
                )

            # ---------------- program ----------------
            alloc_qkv(0)
            alloc_qkv(1)

            # all proj/transpose work as packet blocks, pumped into the
            # attention stream with deadline barriers ((b, j) block tags).
            pre = collections.deque()
            post = collections.deque()  # deferred out-proj

            def add_block(b, j):
                for w_idx in range(3):
                    pre.append(((b, j),
                                lambda bb=b, w=w_idx, jj=j: proj_packet(bb, w, jj)))
                for sc in range(j * 4, (j + 1) * 4):
                    pre.append(((b, j), lambda bb=b, s=sc: v_transpose(bb, s)))

            # block (0,0) inline, d-major interleaved across q/k/v so the PE
            # tracks the xt half0 DMA chunk arrivals
            pq = ps_w.tile([P, NQ], F32, tag="w", name="pp_q0")
            pk = ps_w.tile([P, NQ], F32, tag="w", name="pp_k0")
            pv = ps_s.tile([P, 2, NQ], F32, tag="s", name="pp_v0")
            for d in range(DC):
                for pp, w_sb in ((pq, wq_sb), (pk, wk_sb), (pv, wv_sb)):
                    dst = pp[:, 0, :] if pp is pv else pp[:]
                    nc.tensor.matmul(
                        dst, lhsT=w_sb[:, d, :], rhs=xt_sbs[0][:, d, 0:NQ],
                        start=(d == 0), stop=(d == DC - 1),
                    )
            proj_copy(0, st[0]["qt"][:, 0, :], pq[:], "act")
            proj_copy(1, st[0]["kt"][:, 0, :], pk[:], "act")
            proj_copy(2, st[0]["vt"][:, 0, :], pv[:, 0, :], "act")
            for sc in range(4):
                v_transpose(0, sc)

            for j in range(1, QJ):
                add_block(0, j)
            for j in range(QJ):
                add_block(1, j)

            groups_left = [2 * B * sum(range(1, QJ + 1)) if causal
                           else B * (KC // GK)]
            _ptog = [0]

            def pump():
                groups_left[0] -= 1
                backlog = len(pre) + len(post)
                n = 2 if backlog > groups_left[0] else 1
                for _ in range(n):
                    _ptog[0] ^= 1
                    if pre and (_ptog[0] or not post):
                        pre.popleft()[1]()
                    elif post:
                        post.popleft()()

            def flush_pre(b, qj):
                while pre and pre[0][0][0] == b and pre[0][0][1] <= qj:
                    pre.popleft()[1]()

            attn_alloc(0)
            attn_alloc(1)
            sched = [(0, 0), (0, 1), (0, 2), (0, 3), (1, 0), (1, 1), (1, 2),
                     (1, 3)]
            for i, (b, qj) in enumerate(sched):
                flush_pre(b, qj)
                o_ps = attn_qj(b, qj, pump)
                tail = norm_qj(b, qj, o_ps, last=(i == len(sched) - 1))
                if i < len(sched) - 1:
                    for fn in tail:
                        post.append(fn)
                    for sc in range(qj * 4, (qj + 1) * 4):
                        post.append(lambda bb=b, s=sc: outproj_sc(bb, s))
                else:
                    for fn in tail:
                        fn()

            while pre:
                pre.popleft()[1]()
            while post:
                post.popleft()()
            for sc in range(12, 16):
                outproj_sc(1, sc, engine="act")

    nc.compile()
    _NC_CACHE[key] = nc
    return nc


def _check_causal(mask: np.ndarray) -> bool:
    m = np.asarray(mask).reshape(mask.shape[-2], mask.shape[-1])
    s = m.shape[0]
    if np.array_equal(m, np.tril(np.ones((s, s), dtype=bool))):
        return True
    if m.all():
        return False
    raise NotImplementedError("only causal or all-true masks are supported")


def _warr(w):
    # [D, HH] -> [P, DC, HH] fp16 (partition-major chunks, contiguous DMA)
    return np.ascontiguousarray(
        w.reshape(DC, P, HH).transpose(1, 0, 2).astype(np.float16))


def kernel(inputs_q, mask, Wq, bq, Wk, bk, Wv, bv, Wo, bo, _trace=False,
           _trace_cores=None):
    inputs_q = np.asarray(inputs_q, dtype=np.float32)
    Wq = np.asarray(Wq, dtype=np.float32).reshape(D, H * HD)
    Wk = np.asarray(Wk, dtype=np.float32).reshape(D, H * HD)
    Wv = np.asarray(Wv, dtype=np.float32).reshape(D, H * HD)
    Wo = np.asarray(Wo, dtype=np.float32).reshape(H * HD, D)
    bq = np.asarray(bq, dtype=np.float32).reshape(H * HD)
    bk = np.asarray(bk, dtype=np.float32).reshape(H * HD)
    bv = np.asarray(bv, dtype=np.float32).reshape(H * HD)
    bo = np.asarray(bo, dtype=np.float32).reshape(D)

    causal = _check_causal(mask)
    with_bias_qkv = bool(bq.any() or bk.any() or bv.any())
    with_bias_o = bool(bo.any())

    nc = _build_nc(with_bias_qkv, with_bias_o, causal)

    xt = np.ascontiguousarray(
        inputs_q.transpose(0, 2, 1).astype(np.float16))  # [B, D, S]
    in_maps = []
    for c in range(NCORES):
        f0, f1 = c * HH, (c + 1) * HH
        m = {
            "xt": xt,
            "wq": _warr(Wq[:, f0:f1]),
            "wk": _warr(Wk[:, f0:f1]),
            "wv": _warr(Wv[:, f0:f1]),
            "wo": np.ascontiguousarray(Wo[f0:f1, :].astype(np.float16)),
        }
        if with_bias_qkv:
            m["bqkv"] = np.ascontiguousarray(
                np.stack([bq[f0:f1], bk[f0:f1], bv[f0:f1]])
            )
        if with_bias_o:
            m["bo8"] = np.ascontiguousarray((bo / NCORES).astype(np.float16))
        in_maps.append(m)

    kwargs = {}
    if _trace:
        kwargs["trace"] = True
        if _trace_cores is not None:
            kwargs["trace_cores"] = _trace_cores
    res = run_bass_kernel_spmd(nc, in_maps, core_ids=list(range(NCORES)), **kwargs)

    acc = np.zeros((B, S, D), dtype=np.float32)
    for c in range(NCORES):
        acc += res.results[c]["out"].astype(np.float32)
    if not with_bias_o:
        acc += bo  # bo is zero here, but keep the math explicit
    out = acc.astype(np.float32)
    if _trace:
        return out, res
    return out
